# revision 4
# baseline (speedup 1.0000x reference)
"""Viterbi (CRF decode) kernel for Trainium2, 8 NeuronCores.

Problem: single sequence T=16384, K=35 tags. reference computes a forward
Viterbi pass (max-plus recurrence over time, sequential), backpointers, then
backtracks; returns (path_score, best_path[T]).

Strategy: the time recurrence is chunked into 1024 lanes of 16 steps
(128 lanes per core x 8 cores, lane = one 16-step record window). Each lane
first runs a 32-step warmup from a magnitude-matched warm-start vector so
that, by tropical (max-plus) coalescence, its state equals the true forward
state up to a uniform additive constant that is a multiple of the local
float32 ulp grid -- which makes every subsequent fp32 add/max/argmax
bitwise-identical to the reference's (shift by a grid multiple commutes with
rounding within a binade). The record phase then emits exact backpointers
for its 16 time steps. Host side: a short exact prefix (512 steps) supplies
true warm starts for early lanes plus a drift estimate for the magnitude
guesses, and afterwards backtracks the backpointers and reproduces the
reference's fp32-accumulated path score bitwise by folding along the path.
"""

import numpy as np

K = 35
START = 33
STOP = 34
T = 16384
NEG_INF = np.float32(-10000.0)

W = 32          # warmup steps per lane
L = 16          # recorded steps per lane
P0 = 512        # host-exact prefix length
NCORES = 8
LPC = 128       # lanes per core
C = T // L      # 1024 lanes total

_BIG = float(2.0 ** 30)


def _build_nc():
    import concourse.bass as bass
    import concourse.mybir as mybir

    f32 = mybir.dt.float32
    Alu = mybir.AluOpType
    X = mybir.AxisListType.X

    nc = bass.Bass()

    warm = nc.declare_dram_parameter("warm", [LPC, K], f32, isOutput=False)
    featw = nc.declare_dram_parameter("featw", [LPC, W * K], f32, isOutput=False)
    featr = nc.declare_dram_parameter("featr", [LPC, L * K], f32, isOutput=False)
    transrep = nc.declare_dram_parameter("transrep", [LPC, K * K], f32, isOutput=False)
    iotapm = nc.declare_dram_parameter("iotapm", [LPC, K * K], f32, isOutput=False)
    blend = nc.declare_dram_parameter("blend", [LPC, K], f32, isOutput=False)
    m1 = nc.declare_dram_parameter("m1", [LPC, 1], f32, isOutput=False)
    bp_out = nc.declare_dram_parameter("bp", [LPC, L * K], f32, isOutput=True)
    vend_out = nc.declare_dram_parameter("vend", [LPC, K], f32, isOutput=True)

    with (
        nc.sbuf_tensor([LPC, K * K], f32) as tr,
        nc.sbuf_tensor([LPC, K * K], f32) as io,
        nc.sbuf_tensor([LPC, W * K], f32) as fw,
        nc.sbuf_tensor([LPC, L * K], f32) as fr,
        nc.sbuf_tensor([LPC, K], f32) as bl,
        nc.sbuf_tensor([LPC, 1], f32) as m1t,
        nc.sbuf_tensor([LPC, K], f32) as v,
        nc.sbuf_tensor([LPC, K], f32) as newv,
        nc.sbuf_tensor([LPC, K * K], f32) as cand,
        nc.sbuf_tensor([LPC, K * K], f32) as d,
        nc.sbuf_tensor([LPC, K * K], f32) as keyt,
        nc.sbuf_tensor([LPC, L * K], f32) as bpt,
        nc.semaphore() as dma_sem,
        nc.semaphore() as v_sem,
        nc.Block() as block,
    ):
        tr3 = tr[:].rearrange("x (n p) -> x n p", n=K)
        cand3 = cand[:].rearrange("x (n p) -> x n p", n=K)

        @block.sync
        def _(sync):
            for dst, src in (
                (tr, transrep), (io, iotapm), (fw, featw), (fr, featr),
                (bl, blend), (m1t, m1), (v, warm),
            ):
                sync.dma_start(out=dst[:], in_=src[:]).then_inc(dma_sem, 16)
            sync.wait_ge(v_sem, 1)
            sync.dma_start(out=bp_out[:], in_=bpt[:]).then_inc(dma_sem, 16)
            sync.dma_start(out=vend_out[:], in_=v[:]).then_inc(dma_sem, 16)

        @block.vector
        def _(vector):
            vector.wait_ge(dma_sem, 7 * 16)

            def step(feat_slice, rec_s):
                v_b, tr_b = bass.broadcast_tensor_aps(
                    v[:].rearrange("x (a m) -> x a m", a=1), tr3
                )
                vector.tensor_tensor(out=cand3, in0=v_b, in1=tr_b, op=Alu.add)
                vector.drain()
                vector.tensor_reduce(out=newv[:], in_=cand3, axis=X, op=Alu.max)
                vector.drain()
                if rec_s is not None:
                    # d[p, n] = newv[n] - cand[n, p]   (p-major layout)
                    nv_b, candT = bass.broadcast_tensor_aps(
                        newv[:].rearrange("x (a m) -> x a m", a=1),
                        cand[:].rearrange("x (n p) -> x p n", n=K),
                    )
                    vector.tensor_tensor(
                        out=d[:].rearrange("x (p n) -> x p n", p=K),
                        in0=nv_b, in1=candT, op=Alu.subtract,
                    )
                    vector.drain()
                    # key = d * 2^30 + p ; min over p -> first argmax index
                    vector.scalar_tensor_tensor(
                        out=keyt[:], in0=d[:], scalar=_BIG, in1=io[:],
                        op0=Alu.mult, op1=Alu.add,
                    )
                    vector.drain()
                    vector.tensor_reduce(
                        out=bpt[:, rec_s * K:(rec_s + 1) * K],
                        in_=keyt[:].rearrange("x (p n) -> x n p", p=K),
                        axis=X, op=Alu.min,
                    )
                vector.tensor_tensor(out=v[:], in0=newv[:], in1=feat_slice, op=Alu.add)
                vector.drain()

            for s in range(W):
                step(fw[:, s * K:(s + 1) * K], None)

            # blend: record start = max(v + m1, blend_tile); on core 0 lane 0
            # m1 = -2^17 and blend = init_v, forcing the true initial vector.
            vector.tensor_scalar_add(out=newv[:], in0=v[:], scalar1=m1t[:, 0:1])
            vector.drain()
            vector.tensor_tensor(out=v[:], in0=newv[:], in1=bl[:], op=Alu.max)
            vector.drain()

            for s in range(L):
                step(fr[:, s * K:(s + 1) * K], s)

            vector.drain().then_inc(v_sem, 1)
    return nc


def _host_inputs(feats, trans):
    """Build per-core input dicts. feats [T,K] f32, trans [K,K] f32."""
    feats = np.ascontiguousarray(feats, dtype=np.float32)
    trans = np.ascontiguousarray(trans, dtype=np.float32)

    # exact fp32 prefix (reference arithmetic) for warm starts + drift rate
    v = np.full(K, NEG_INF, dtype=np.float32)
    v[START] = np.float32(0.0)
    vs = np.empty((P0 + 1, K), np.float32)
    vs[0] = v
    for t in range(P0):
        v = (v[None, :] + trans).max(axis=1) + feats[t]
        vs[t + 1] = v
    lam = float(vs[P0].max()) / P0

    ws = np.zeros((C, K), np.float32)
    for c in range(C):
        t0c = L * c - W
        if t0c < 0:
            ws[c] = 0.0
        elif t0c <= P0:
            ws[c] = vs[t0c]
        else:
            ws[c] = np.float32(lam * t0c)

    F = np.concatenate([np.zeros((W, K), np.float32), feats], axis=0)
    # lane c: warmup rows F[L*c : L*c+W], record rows F[L*c+W : L*c+W+L]
    sl = np.lib.stride_tricks.sliding_window_view(F, (W + L, K), axis=(0, 1))
    lanes = sl[np.arange(C) * L, 0]            # [C, W+L, K]
    featw_all = lanes[:, :W].reshape(C, W * K)
    featr_all = lanes[:, W:].reshape(C, L * K)

    init_v = np.full(K, NEG_INF, np.float32)
    init_v[START] = np.float32(0.0)

    transrep = np.broadcast_to(trans.reshape(1, K * K), (LPC, K * K))
    iotapm = np.broadcast_to(
        np.repeat(np.arange(K, dtype=np.float32), K).reshape(1, K * K), (LPC, K * K)
    )

    in_maps = []
    for k in range(NCORES):
        lanes_k = slice(k * LPC, (k + 1) * LPC)
        blend_k = np.full((LPC, K), np.float32(-65536.0))
        m1_k = np.zeros((LPC, 1), np.float32)
        if k == 0:
            blend_k[0] = init_v
            m1_k[0] = np.float32(-131072.0)
        in_maps.append({
            "warm": np.ascontiguousarray(ws[lanes_k]),
            "featw": np.ascontiguousarray(featw_all[lanes_k]),
            "featr": np.ascontiguousarray(featr_all[lanes_k]),
            "transrep": np.ascontiguousarray(transrep),
            "iotapm": np.ascontiguousarray(iotapm),
            "blend": blend_k,
            "m1": m1_k,
        })
    return in_maps


def _postprocess(feats, trans, bp_f32, vend_last):
    """bp_f32 [T, K] float backpointers, vend_last [K] final lane state."""
    bp = bp_f32.astype(np.int32)
    terminal = vend_last + trans[STOP]
    best_last = int(np.argmax(terminal))

    # vectorized backtrack: compose within 16-step chunks, then chain chunks
    bpc = bp.reshape(C, L, K)
    comp = np.broadcast_to(np.arange(K, dtype=np.int32), (C, K)).copy()
    lane_i = np.arange(C)
    for s in range(L - 1, -1, -1):
        comp = bpc[lane_i[:, None], s, comp]   # comp[c, e] = state before step s
    # entry[c] = state at time 16c - 1 given exit state at time 16c+15
    exits = np.empty(C, np.int32)
    e = best_last
    for c in range(C - 1, -1, -1):
        exits[c] = e
        e = comp[c, e]
    path = np.empty((C, L), np.int32)
    cur = exits
    for s in range(L - 1, -1, -1):
        path[:, s] = cur
        cur = bpc[lane_i, s, cur]
    path = path.reshape(T)

    # reproduce the reference's fp32 left-fold score bitwise along the path
    prev = np.concatenate([[START], path[:-1]])
    tstep = trans[path, prev]                  # f32 [T]
    fstep = feats[np.arange(T), path]          # f32 [T]
    acc = np.float32(0.0)
    f32 = np.float32
    for a, b in zip(tstep.tolist(), fstep.tolist()):
        acc = f32(f32(acc + f32(a)) + f32(b))
    score = f32(acc + trans[STOP, path[-1]])
    return score, path


_last_results = None  # BassKernelResults of the most recent run (for profiling)


def kernel(feats, transitions):
    global _last_results
    feats = np.ascontiguousarray(feats, dtype=np.float32)
    trans = np.ascontiguousarray(transitions, dtype=np.float32)

    from concourse.bass_utils import run_bass_kernel_spmd

    nc = _build_nc()
    in_maps = _host_inputs(feats, trans)
    res = run_bass_kernel_spmd(nc, in_maps, list(range(NCORES)))
    _last_results = res

    bp_f32 = np.empty((T, K), np.float32)
    for k in range(NCORES):
        bp_k = np.asarray(res.results[k]["bp"]).reshape(LPC, L, K)
        t0k = 16 * (k * LPC)
        bp_f32[t0k:t0k + LPC * L] = bp_k.reshape(LPC * L, K)
    vend_last = np.asarray(res.results[NCORES - 1]["vend"])[LPC - 1]

    score, path = _postprocess(feats, trans, bp_f32, vend_last)
    return score, path.astype(np.int32)


# revision 7
# speedup vs baseline: 1.2558x; 1.2558x over previous
"""Viterbi (CRF decode) kernel for Trainium2, 8 NeuronCores.

Problem: single sequence T=16384, K=35 tags. reference computes a forward
Viterbi pass (max-plus recurrence over time, sequential), backpointers, then
backtracks; returns (path_score, best_path[T]).

Strategy: the time recurrence is chunked into 1024 lanes of 16 steps
(128 lanes per core x 8 cores, lane = one 16-step record window). Each lane
first runs a 32-step warmup from a magnitude-matched warm-start vector so
that, by tropical (max-plus) coalescence, its state equals the true forward
state up to a uniform additive constant that is a multiple of the local
float32 ulp grid -- which makes every subsequent fp32 add/max/argmax
bitwise-identical to the reference's (shift by a grid multiple commutes with
rounding within a binade). The record phase then emits exact backpointers
for its 16 time steps. Host side: a short exact prefix (512 steps) supplies
true warm starts for early lanes plus a drift estimate for the magnitude
guesses, and afterwards backtracks the backpointers and reproduces the
reference's fp32-accumulated path score bitwise by folding along the path.
"""

import numpy as np

K = 35
START = 33
STOP = 34
T = 16384
NEG_INF = np.float32(-10000.0)

W = 24          # warmup steps per lane
L = 16          # recorded steps per lane
P0 = 512        # host-exact prefix length
NCORES = 8
LPC = 128       # lanes per core
C = T // L      # 1024 lanes total

_BIG = float(2.0 ** 30)


def _build_nc():
    import concourse.bass as bass
    import concourse.mybir as mybir

    f32 = mybir.dt.float32
    Alu = mybir.AluOpType
    X = mybir.AxisListType.X

    nc = bass.Bass()

    warm = nc.declare_dram_parameter("warm", [LPC, K], f32, isOutput=False)
    featw = nc.declare_dram_parameter("featw", [LPC, W * K], f32, isOutput=False)
    featr = nc.declare_dram_parameter("featr", [LPC, L * K], f32, isOutput=False)
    transrep = nc.declare_dram_parameter("transrep", [LPC, K * K], f32, isOutput=False)
    iotapm = nc.declare_dram_parameter("iotapm", [LPC, K * K], f32, isOutput=False)
    blend = nc.declare_dram_parameter("blend", [LPC, K], f32, isOutput=False)
    m1 = nc.declare_dram_parameter("m1", [LPC, 1], f32, isOutput=False)
    bp_out = nc.declare_dram_parameter("bp", [LPC, L * K], f32, isOutput=True)
    vend_out = nc.declare_dram_parameter("vend", [LPC, K], f32, isOutput=True)

    with (
        nc.sbuf_tensor([LPC, K * K], f32) as tr,
        nc.sbuf_tensor([LPC, K * K], f32) as io,
        nc.sbuf_tensor([LPC, W * K], f32) as fw,
        nc.sbuf_tensor([LPC, L * K], f32) as fr,
        nc.sbuf_tensor([LPC, K], f32) as bl,
        nc.sbuf_tensor([LPC, 1], f32) as m1t,
        nc.sbuf_tensor([LPC, K], f32) as v,
        nc.sbuf_tensor([LPC, K], f32) as newv,
        nc.sbuf_tensor([LPC, K * K], f32) as cand,
        nc.sbuf_tensor([LPC, K * K], f32) as d,
        nc.sbuf_tensor([LPC, K * K], f32) as keyt,
        nc.sbuf_tensor([LPC, L * K], f32) as bpt,
        nc.semaphore() as dma_sem,
        nc.semaphore() as v_sem,
        nc.Block() as block,
    ):
        tr3 = tr[:].rearrange("x (n p) -> x n p", n=K)
        cand3 = cand[:].rearrange("x (n p) -> x n p", n=K)

        @block.sync
        def _(sync):
            for dst, src in (
                (tr, transrep), (io, iotapm), (fw, featw), (fr, featr),
                (bl, blend), (m1t, m1), (v, warm),
            ):
                sync.dma_start(out=dst[:], in_=src[:]).then_inc(dma_sem, 16)
            sync.wait_ge(v_sem, 1)
            sync.dma_start(out=bp_out[:], in_=bpt[:]).then_inc(dma_sem, 16)
            sync.dma_start(out=vend_out[:], in_=v[:]).then_inc(dma_sem, 16)

        @block.vector
        def _(vector):
            vector.wait_ge(dma_sem, 7 * 16)

            def step(feat_slice, rec_s):
                v_b, tr_b = bass.broadcast_tensor_aps(
                    v[:].rearrange("x (a m) -> x a m", a=1), tr3
                )
                vector.tensor_tensor(out=cand3, in0=v_b, in1=tr_b, op=Alu.add)
                vector.drain()
                vector.tensor_reduce(out=newv[:], in_=cand3, axis=X, op=Alu.max)
                vector.drain()
                if rec_s is not None:
                    # d[n, p] = newv[n] - cand[n, p]   (n-major, contiguous)
                    nv_b, cand_b = bass.broadcast_tensor_aps(
                        newv[:].rearrange("x (m a) -> x m a", a=1), cand3
                    )
                    vector.tensor_tensor(
                        out=d[:].rearrange("x (n p) -> x n p", n=K),
                        in0=nv_b, in1=cand_b, op=Alu.subtract,
                    )
                    vector.drain()
                    # key = d * 2^30 + p ; min over p -> first argmax index
                    vector.scalar_tensor_tensor(
                        out=keyt[:], in0=d[:], scalar=_BIG, in1=io[:],
                        op0=Alu.mult, op1=Alu.add,
                    )
                    vector.drain()
                    vector.tensor_reduce(
                        out=bpt[:, rec_s * K:(rec_s + 1) * K],
                        in_=keyt[:].rearrange("x (n p) -> x n p", n=K),
                        axis=X, op=Alu.min,
                    )
                vector.tensor_tensor(out=v[:], in0=newv[:], in1=feat_slice, op=Alu.add)
                vector.drain()

            for s in range(W):
                step(fw[:, s * K:(s + 1) * K], None)

            # blend: record start = max(v + m1, blend_tile); on core 0 lane 0
            # m1 = -2^17 and blend = init_v, forcing the true initial vector.
            vector.tensor_scalar_add(out=newv[:], in0=v[:], scalar1=m1t[:, 0:1])
            vector.drain()
            vector.tensor_tensor(out=v[:], in0=newv[:], in1=bl[:], op=Alu.max)
            vector.drain()

            for s in range(L):
                step(fr[:, s * K:(s + 1) * K], s)

            vector.drain().then_inc(v_sem, 1)
    return nc


def _host_inputs(feats, trans):
    """Build per-core input dicts. feats [T,K] f32, trans [K,K] f32."""
    feats = np.ascontiguousarray(feats, dtype=np.float32)
    trans = np.ascontiguousarray(trans, dtype=np.float32)

    # exact fp32 prefix (reference arithmetic) for warm starts + drift rate
    v = np.full(K, NEG_INF, dtype=np.float32)
    v[START] = np.float32(0.0)
    vs = np.empty((P0 + 1, K), np.float32)
    vs[0] = v
    for t in range(P0):
        v = (v[None, :] + trans).max(axis=1) + feats[t]
        vs[t + 1] = v
    lam = float(vs[P0].max()) / P0

    ws = np.zeros((C, K), np.float32)
    for c in range(C):
        t0c = L * c - W
        if t0c < 0:
            ws[c] = 0.0
        elif t0c <= P0:
            ws[c] = vs[t0c]
        else:
            ws[c] = np.float32(lam * t0c)

    F = np.concatenate([np.zeros((W, K), np.float32), feats], axis=0)
    # lane c: warmup rows F[L*c : L*c+W], record rows F[L*c+W : L*c+W+L]
    sl = np.lib.stride_tricks.sliding_window_view(F, (W + L, K), axis=(0, 1))
    lanes = sl[np.arange(C) * L, 0]            # [C, W+L, K]
    featw_all = lanes[:, :W].reshape(C, W * K)
    featr_all = lanes[:, W:].reshape(C, L * K)

    init_v = np.full(K, NEG_INF, np.float32)
    init_v[START] = np.float32(0.0)

    transrep = np.broadcast_to(trans.reshape(1, K * K), (LPC, K * K))
    # n-major iota: value at col n*K+p is p
    iotapm = np.broadcast_to(
        np.tile(np.arange(K, dtype=np.float32), K).reshape(1, K * K), (LPC, K * K)
    )

    in_maps = []
    for k in range(NCORES):
        lanes_k = slice(k * LPC, (k + 1) * LPC)
        blend_k = np.full((LPC, K), np.float32(-65536.0))
        m1_k = np.zeros((LPC, 1), np.float32)
        if k == 0:
            blend_k[0] = init_v
            m1_k[0] = np.float32(-131072.0)
        in_maps.append({
            "warm": np.ascontiguousarray(ws[lanes_k]),
            "featw": np.ascontiguousarray(featw_all[lanes_k]),
            "featr": np.ascontiguousarray(featr_all[lanes_k]),
            "transrep": np.ascontiguousarray(transrep),
            "iotapm": np.ascontiguousarray(iotapm),
            "blend": blend_k,
            "m1": m1_k,
        })
    return in_maps


def _postprocess(feats, trans, bp_f32, vend_last):
    """bp_f32 [T, K] float backpointers, vend_last [K] final lane state."""
    bp = bp_f32.astype(np.int32)
    terminal = vend_last + trans[STOP]
    best_last = int(np.argmax(terminal))

    # vectorized backtrack: compose within 16-step chunks, then chain chunks
    bpc = bp.reshape(C, L, K)
    comp = np.broadcast_to(np.arange(K, dtype=np.int32), (C, K)).copy()
    lane_i = np.arange(C)
    for s in range(L - 1, -1, -1):
        comp = bpc[lane_i[:, None], s, comp]   # comp[c, e] = state before step s
    # entry[c] = state at time 16c - 1 given exit state at time 16c+15
    exits = np.empty(C, np.int32)
    e = best_last
    for c in range(C - 1, -1, -1):
        exits[c] = e
        e = comp[c, e]
    path = np.empty((C, L), np.int32)
    cur = exits
    for s in range(L - 1, -1, -1):
        path[:, s] = cur
        cur = bpc[lane_i, s, cur]
    path = path.reshape(T)

    # reproduce the reference's fp32 left-fold score bitwise along the path
    prev = np.concatenate([[START], path[:-1]])
    tstep = trans[path, prev]                  # f32 [T]
    fstep = feats[np.arange(T), path]          # f32 [T]
    acc = np.float32(0.0)
    f32 = np.float32
    for a, b in zip(tstep.tolist(), fstep.tolist()):
        acc = f32(f32(acc + f32(a)) + f32(b))
    score = f32(acc + trans[STOP, path[-1]])
    return score, path


_last_results = None  # BassKernelResults of the most recent run (for profiling)


def kernel(feats, transitions):
    global _last_results
    feats = np.ascontiguousarray(feats, dtype=np.float32)
    trans = np.ascontiguousarray(transitions, dtype=np.float32)

    from concourse.bass_utils import run_bass_kernel_spmd

    nc = _build_nc()
    in_maps = _host_inputs(feats, trans)
    res = run_bass_kernel_spmd(nc, in_maps, list(range(NCORES)))
    _last_results = res

    bp_f32 = np.empty((T, K), np.float32)
    for k in range(NCORES):
        bp_k = np.asarray(res.results[k]["bp"]).reshape(LPC, L, K)
        t0k = 16 * (k * LPC)
        bp_f32[t0k:t0k + LPC * L] = bp_k.reshape(LPC * L, K)
    vend_last = np.asarray(res.results[NCORES - 1]["vend"])[LPC - 1]

    score, path = _postprocess(feats, trans, bp_f32, vend_last)
    return score, path.astype(np.int32)


# revision 24
# speedup vs baseline: 1.3811x; 1.0998x over previous
"""Viterbi (CRF decode) kernel for Trainium2, 8 NeuronCores.

Problem: single sequence T=16384, K=35 tags. reference computes a forward
Viterbi pass (max-plus recurrence over time, sequential), backpointers, then
backtracks; returns (path_score, best_path[T]).

Strategy: the time recurrence is chunked into 1024 lanes of 16 steps
(128 lanes per core x 8 cores, lane = one 16-step record window). Each lane
first runs a 32-step warmup from a magnitude-matched warm-start vector so
that, by tropical (max-plus) coalescence, its state equals the true forward
state up to a uniform additive constant that is a multiple of the local
float32 ulp grid -- which makes every subsequent fp32 add/max/argmax
bitwise-identical to the reference's (shift by a grid multiple commutes with
rounding within a binade). The record phase then emits exact backpointers
for its 16 time steps. Host side: a short exact prefix (512 steps) supplies
true warm starts for early lanes plus a drift estimate for the magnitude
guesses, and afterwards backtracks the backpointers and reproduces the
reference's fp32-accumulated path score bitwise by folding along the path.
"""

import numpy as np

K = 35
START = 33
STOP = 34
T = 16384
NEG_INF = np.float32(-10000.0)

W = 24          # warmup steps per lane
L = 16          # recorded steps per lane
P0 = 512        # host-exact prefix length
NCORES = 8
LPC = 128       # lanes per core
C = T // L      # 1024 lanes total

_BIG = float(2.0 ** 30)
_EPS = np.float32(2.0 ** -120)  # index packing scale: key = d + p*EPS


def _build_nc():
    import concourse.bass as bass
    import concourse.bacc as bacc
    import concourse.mybir as mybir

    f32 = mybir.dt.float32
    Alu = mybir.AluOpType
    X = mybir.AxisListType.X

    nc = bacc.Bacc()

    warm = nc.declare_dram_parameter("warm", [LPC, K], f32, isOutput=False)
    featw = nc.declare_dram_parameter("featw", [LPC, W * K], f32, isOutput=False)
    featr = nc.declare_dram_parameter("featr", [LPC, L * K], f32, isOutput=False)
    transrep = nc.declare_dram_parameter("transrep", [LPC, K * K], f32, isOutput=False)
    iotapm = nc.declare_dram_parameter("iotapm", [LPC, K * K], f32, isOutput=False)
    blend = nc.declare_dram_parameter("blend", [LPC, K], f32, isOutput=False)
    m1 = nc.declare_dram_parameter("m1", [LPC, 1], f32, isOutput=False)
    bp_out = nc.declare_dram_parameter("bp", [LPC, L * K], f32, isOutput=True)
    vend_out = nc.declare_dram_parameter("vend", [LPC, K], f32, isOutput=True)

    XV = 10  # bp columns n < XV on Vector; n >= XV on GpSimd

    from contextlib import ExitStack

    with ExitStack() as ctx:
        tr = ctx.enter_context(nc.sbuf_tensor([LPC, K * K], f32))
        io = ctx.enter_context(nc.sbuf_tensor([LPC, K * K], f32))
        fw = ctx.enter_context(nc.sbuf_tensor([LPC, W * K], f32))
        fr = ctx.enter_context(nc.sbuf_tensor([LPC, L * K], f32))
        bl = ctx.enter_context(nc.sbuf_tensor([LPC, K], f32))
        m1t = ctx.enter_context(nc.sbuf_tensor([LPC, 1], f32))
        v = ctx.enter_context(nc.sbuf_tensor([LPC, K], f32))
        newv_a = ctx.enter_context(nc.sbuf_tensor([LPC, K], f32))
        newv_b = ctx.enter_context(nc.sbuf_tensor([LPC, K], f32))
        cand_a = ctx.enter_context(nc.sbuf_tensor([LPC, K * K], f32))
        cand_b = ctx.enter_context(nc.sbuf_tensor([LPC, K * K], f32))
        dg = ctx.enter_context(nc.sbuf_tensor([LPC, K * K], f32))
        keyg_a = ctx.enter_context(nc.sbuf_tensor([LPC, K * K], f32))
        keyg_b = ctx.enter_context(nc.sbuf_tensor([LPC, K * K], f32))
        bpt = ctx.enter_context(nc.sbuf_tensor([LPC, L * K], f32))
        dma_sem = ctx.enter_context(nc.semaphore())
        v_sem = ctx.enter_context(nc.semaphore())
        vsem = ctx.enter_context(nc.semaphore())   # vector -> gpsimd: step ready
        grel = ctx.enter_context(nc.semaphore())   # gpsimd -> vector: cand/newv consumed
        gkey = ctx.enter_context(nc.semaphore())   # gpsimd -> vector: key ready
        block = ctx.enter_context(nc.Block())
        cands = [cand_a, cand_b]
        newvs = [newv_a, newv_b]
        keygs = [keyg_a, keyg_b]

        def c3(t, lo=0, hi=K):
            return t[:, lo * K:hi * K].rearrange("x (n p) -> x n p", n=hi - lo)

        @block.sync
        def _(sync):
            for dst, src in (
                (tr, transrep), (io, iotapm), (fw, featw), (fr, featr),
                (bl, blend), (m1t, m1), (v, warm),
            ):
                sync.dma_start(out=dst[:], in_=src[:]).then_inc(dma_sem, 16)
            sync.wait_ge(v_sem, 1)
            sync.dma_start(out=bp_out[:], in_=bpt[:]).then_inc(dma_sem, 16)
            sync.dma_start(out=vend_out[:], in_=v[:]).then_inc(dma_sem, 16)

        @block.vector
        def _(vector):
            vector.wait_ge(dma_sem, 7 * 16)

            def rmin(rec_s):
                # stored bp[rec_s] = min over p of (d + p*EPS) = first_argmax * EPS
                vector.wait_ge(gkey, rec_s + 1)
                vector.tensor_reduce(
                    out=bpt[:, rec_s * K:(rec_s + 1) * K],
                    in_=c3(keygs[rec_s % 2]),
                    axis=X, op=Alu.min,
                )

            def step(feat_slice, rec_s):
                i = (rec_s or 0) % 2
                cand, newv = (cands[i], newvs[i]) if rec_s is not None else (cands[0], newvs[0])
                if rec_s is not None and rec_s >= 2:
                    # gpsimd must be done reading cand/newv of step rec_s-2
                    vector.wait_ge(grel, rec_s - 1)
                cand3 = c3(cand)
                v_b, tr_b = bass.broadcast_tensor_aps(
                    v[:].rearrange("x (a m) -> x a m", a=1),
                    tr[:].rearrange("x (n p) -> x n p", n=K),
                )
                vector.tensor_tensor(out=cand3, in0=v_b, in1=tr_b, op=Alu.add)
                vector.drain()
                vector.tensor_reduce(out=newv[:], in_=cand3, axis=X, op=Alu.max)
                if rec_s is not None:
                    vector.drain().then_inc(vsem, 1)
                else:
                    vector.drain()
                vector.tensor_tensor(out=v[:], in0=newv[:], in1=feat_slice, op=Alu.add)
                if rec_s is not None and rec_s >= 1:
                    rmin(rec_s - 1)   # previous step's argmin, pairs with feat add
                vector.drain()

            for s in range(W):
                step(fw[:, s * K:(s + 1) * K], None)

            # blend: record start = max(v + m1, blend_tile); on core 0 lane 0
            # m1 = -2^17 and blend = init_v, forcing the true initial vector.
            vector.tensor_scalar_add(out=newv_a[:], in0=v[:], scalar1=m1t[:, 0:1])
            vector.drain()
            vector.tensor_tensor(out=v[:], in0=newv_a[:], in1=bl[:], op=Alu.max)
            vector.drain()

            for s in range(L):
                step(fr[:, s * K:(s + 1) * K], s)

            rmin(L - 1)
            vector.drain().then_inc(v_sem, 1)

        @block.gpsimd
        def _(gpsimd):
            from concourse import library_config
            gpsimd.load_library(library_config.standard)
            for s in range(L):
                i = s % 2
                cand, newv = cands[i], newvs[i]
                gpsimd.wait_ge(vsem, s + 1)
                # d[n, p] = newv[n] - cand[n, p]  (exactly 0.0 at the argmax set)
                nv_b, cand_b_ = bass.broadcast_tensor_aps(
                    newv[:].rearrange("x (m a) -> x m a", a=1), c3(cand)
                )
                gpsimd.tensor_tensor(
                    out=c3(dg), in0=nv_b, in1=cand_b_, op=Alu.subtract,
                )
                gpsimd.drain().then_inc(grel, 1)
                # key = d + p*EPS: the tiny index term survives only where d == 0
                gpsimd.tensor_tensor(
                    out=keygs[i][:], in0=dg[:], in1=io[:], op=Alu.add,
                )
                gpsimd.drain().then_inc(gkey, 1)

    nc.compile()
    return nc


def _host_inputs(feats, trans):
    """Build per-core input dicts. feats [T,K] f32, trans [K,K] f32."""
    feats = np.ascontiguousarray(feats, dtype=np.float32)
    trans = np.ascontiguousarray(trans, dtype=np.float32)

    # exact fp32 prefix (reference arithmetic) for warm starts + drift rate
    v = np.full(K, NEG_INF, dtype=np.float32)
    v[START] = np.float32(0.0)
    vs = np.empty((P0 + 1, K), np.float32)
    vs[0] = v
    for t in range(P0):
        v = (v[None, :] + trans).max(axis=1) + feats[t]
        vs[t + 1] = v
    lam = float(vs[P0].max()) / P0

    ws = np.zeros((C, K), np.float32)
    for c in range(C):
        t0c = L * c - W
        if t0c < 0:
            ws[c] = 0.0
        elif t0c <= P0:
            ws[c] = vs[t0c]
        else:
            ws[c] = np.float32(lam * t0c)

    F = np.concatenate([np.zeros((W, K), np.float32), feats], axis=0)
    # lane c: warmup rows F[L*c : L*c+W], record rows F[L*c+W : L*c+W+L]
    sl = np.lib.stride_tricks.sliding_window_view(F, (W + L, K), axis=(0, 1))
    lanes = sl[np.arange(C) * L, 0]            # [C, W+L, K]
    featw_all = lanes[:, :W].reshape(C, W * K)
    featr_all = lanes[:, W:].reshape(C, L * K)

    init_v = np.full(K, NEG_INF, np.float32)
    init_v[START] = np.float32(0.0)

    transrep = np.broadcast_to(trans.reshape(1, K * K), (LPC, K * K))
    # n-major packed index: value at col n*K+p is p * EPS
    iotapm = np.broadcast_to(
        (_EPS * np.tile(np.arange(K, dtype=np.float32), K)).reshape(1, K * K),
        (LPC, K * K),
    )

    in_maps = []
    for k in range(NCORES):
        lanes_k = slice(k * LPC, (k + 1) * LPC)
        blend_k = np.full((LPC, K), np.float32(-65536.0))
        m1_k = np.zeros((LPC, 1), np.float32)
        if k == 0:
            blend_k[0] = init_v
            m1_k[0] = np.float32(-131072.0)
        in_maps.append({
            "warm": np.ascontiguousarray(ws[lanes_k]),
            "featw": np.ascontiguousarray(featw_all[lanes_k]),
            "featr": np.ascontiguousarray(featr_all[lanes_k]),
            "transrep": np.ascontiguousarray(transrep),
            "iotapm": np.ascontiguousarray(iotapm),
            "blend": blend_k,
            "m1": m1_k,
        })
    return in_maps


def _postprocess(feats, trans, bp_f32, vend_last):
    """bp_f32 [T, K] stored as index * EPS; vend_last [K] final lane state."""
    bp = np.rint(bp_f32.astype(np.float64) * float(2.0 ** 120)).astype(np.int32)
    terminal = vend_last + trans[STOP]
    best_last = int(np.argmax(terminal))

    # vectorized backtrack: compose within 16-step chunks, then chain chunks
    bpc = bp.reshape(C, L, K)
    comp = np.broadcast_to(np.arange(K, dtype=np.int32), (C, K)).copy()
    lane_i = np.arange(C)
    for s in range(L - 1, -1, -1):
        comp = bpc[lane_i[:, None], s, comp]   # comp[c, e] = state before step s
    # entry[c] = state at time 16c - 1 given exit state at time 16c+15
    exits = np.empty(C, np.int32)
    e = best_last
    for c in range(C - 1, -1, -1):
        exits[c] = e
        e = comp[c, e]
    path = np.empty((C, L), np.int32)
    cur = exits
    for s in range(L - 1, -1, -1):
        path[:, s] = cur
        cur = bpc[lane_i, s, cur]
    path = path.reshape(T)

    # reproduce the reference's fp32 left-fold score bitwise along the path
    prev = np.concatenate([[START], path[:-1]])
    tstep = trans[path, prev]                  # f32 [T]
    fstep = feats[np.arange(T), path]          # f32 [T]
    acc = np.float32(0.0)
    f32 = np.float32
    for a, b in zip(tstep.tolist(), fstep.tolist()):
        acc = f32(f32(acc + f32(a)) + f32(b))
    score = f32(acc + trans[STOP, path[-1]])
    return score, path


_last_results = None  # BassKernelResults of the most recent run (for profiling)


def kernel(feats, transitions):
    global _last_results
    feats = np.ascontiguousarray(feats, dtype=np.float32)
    trans = np.ascontiguousarray(transitions, dtype=np.float32)

    from concourse.bass_utils import run_bass_kernel_spmd

    nc = _build_nc()
    in_maps = _host_inputs(feats, trans)
    res = run_bass_kernel_spmd(nc, in_maps, list(range(NCORES)))
    _last_results = res

    bp_f32 = np.empty((T, K), np.float32)
    for k in range(NCORES):
        bp_k = np.asarray(res.results[k]["bp"]).reshape(LPC, L, K)
        t0k = 16 * (k * LPC)
        bp_f32[t0k:t0k + LPC * L] = bp_k.reshape(LPC * L, K)
    vend_last = np.asarray(res.results[NCORES - 1]["vend"])[LPC - 1]

    score, path = _postprocess(feats, trans, bp_f32, vend_last)
    return score, path.astype(np.int32)


# revision 28
# speedup vs baseline: 1.4038x; 1.0164x over previous
"""Viterbi (CRF decode) kernel for Trainium2, 8 NeuronCores.

Problem: single sequence T=16384, K=35 tags. reference computes a forward
Viterbi pass (max-plus recurrence over time, sequential), backpointers, then
backtracks; returns (path_score, best_path[T]).

Strategy: the time recurrence is chunked into 1024 lanes of 16 steps
(128 lanes per core x 8 cores, lane = one 16-step record window). Each lane
first runs a 32-step warmup from a magnitude-matched warm-start vector so
that, by tropical (max-plus) coalescence, its state equals the true forward
state up to a uniform additive constant that is a multiple of the local
float32 ulp grid -- which makes every subsequent fp32 add/max/argmax
bitwise-identical to the reference's (shift by a grid multiple commutes with
rounding within a binade). The record phase then emits exact backpointers
for its 16 time steps. Host side: a short exact prefix (512 steps) supplies
true warm starts for early lanes plus a drift estimate for the magnitude
guesses, and afterwards backtracks the backpointers and reproduces the
reference's fp32-accumulated path score bitwise by folding along the path.
"""

import numpy as np

K = 35
START = 33
STOP = 34
T = 16384
NEG_INF = np.float32(-10000.0)

W = 24          # warmup steps per lane
L = 16          # recorded steps per lane
P0 = 512        # host-exact prefix length
NCORES = 8
LPC = 128       # lanes per core
C = T // L      # 1024 lanes total

_BIG = float(2.0 ** 30)
_EPS = np.float32(2.0 ** -120)  # index packing scale: key = d + p*EPS


def _build_nc():
    import concourse.bass as bass
    import concourse.bacc as bacc
    import concourse.mybir as mybir

    f32 = mybir.dt.float32
    Alu = mybir.AluOpType
    X = mybir.AxisListType.X

    nc = bacc.Bacc(detect_race_conditions=False)

    warm = nc.declare_dram_parameter("warm", [LPC, K], f32, isOutput=False)
    featw = nc.declare_dram_parameter("featw", [LPC, W * K], f32, isOutput=False)
    featr = nc.declare_dram_parameter("featr", [LPC, L * K], f32, isOutput=False)
    transrep = nc.declare_dram_parameter("transrep", [LPC, K * K], f32, isOutput=False)
    iotapm = nc.declare_dram_parameter("iotapm", [LPC, K], f32, isOutput=False)
    blend = nc.declare_dram_parameter("blend", [LPC, K], f32, isOutput=False)
    m1 = nc.declare_dram_parameter("m1", [LPC, 1], f32, isOutput=False)
    bp_out = nc.declare_dram_parameter("bp", [LPC, L * K], f32, isOutput=True)
    vend_out = nc.declare_dram_parameter("vend", [LPC, K], f32, isOutput=True)

    XV = 10  # bp columns n < XV on Vector; n >= XV on GpSimd

    from contextlib import ExitStack

    with ExitStack() as ctx:
        tr = ctx.enter_context(nc.sbuf_tensor([LPC, K * K], f32))
        io = ctx.enter_context(nc.sbuf_tensor([LPC, K], f32))
        fw = ctx.enter_context(nc.sbuf_tensor([LPC, W * K], f32))
        fr = ctx.enter_context(nc.sbuf_tensor([LPC, L * K], f32))
        bl = ctx.enter_context(nc.sbuf_tensor([LPC, K], f32))
        m1t = ctx.enter_context(nc.sbuf_tensor([LPC, 1], f32))
        v = ctx.enter_context(nc.sbuf_tensor([LPC, K], f32))
        newv_a = ctx.enter_context(nc.sbuf_tensor([LPC, K], f32))
        newv_b = ctx.enter_context(nc.sbuf_tensor([LPC, K], f32))
        cand_a = ctx.enter_context(nc.sbuf_tensor([LPC, K * K], f32))
        cand_b = ctx.enter_context(nc.sbuf_tensor([LPC, K * K], f32))
        dg = ctx.enter_context(nc.sbuf_tensor([LPC, K * K], f32))
        keyg_a = ctx.enter_context(nc.sbuf_tensor([LPC, K * K], f32))
        keyg_b = ctx.enter_context(nc.sbuf_tensor([LPC, K * K], f32))
        bpt = ctx.enter_context(nc.sbuf_tensor([LPC, L * K], f32))
        dma_sem = ctx.enter_context(nc.semaphore())
        v_sem = ctx.enter_context(nc.semaphore())
        vsem = ctx.enter_context(nc.semaphore())   # vector -> gpsimd: step ready
        grel = ctx.enter_context(nc.semaphore())   # gpsimd -> vector: cand/newv consumed
        gkey = ctx.enter_context(nc.semaphore())   # gpsimd -> vector: key ready
        block = ctx.enter_context(nc.Block())
        cands = [cand_a, cand_b]
        newvs = [newv_a, newv_b]
        keygs = [keyg_a, keyg_b]

        def c3(t, lo=0, hi=K):
            return t[:, lo * K:hi * K].rearrange("x (n p) -> x n p", n=hi - lo)

        @block.sync
        def _(sync):
            for dst, src in (
                (tr, transrep), (io, iotapm), (fw, featw), (fr, featr),
                (bl, blend), (m1t, m1), (v, warm),
            ):
                sync.dma_start(out=dst[:], in_=src[:]).then_inc(dma_sem, 16)
            sync.wait_ge(v_sem, 1)
            sync.dma_start(out=bp_out[:], in_=bpt[:]).then_inc(dma_sem, 16)
            sync.dma_start(out=vend_out[:], in_=v[:]).then_inc(dma_sem, 16)

        @block.vector
        def _(vector):
            vector.wait_ge(dma_sem, 7 * 16)

            def rmin(rec_s):
                # stored bp[rec_s] = min over p of (d + p*EPS) = first_argmax * EPS
                vector.wait_ge(gkey, rec_s + 1)
                vector.tensor_reduce(
                    out=bpt[:, rec_s * K:(rec_s + 1) * K],
                    in_=c3(keygs[rec_s % 2]),
                    axis=X, op=Alu.min,
                )

            def step(feat_slice, rec_s):
                i = (rec_s or 0) % 2
                cand, newv = (cands[i], newvs[i]) if rec_s is not None else (cands[0], newvs[0])
                if rec_s is not None and rec_s >= 2:
                    # gpsimd must be done reading cand/newv of step rec_s-2
                    vector.wait_ge(grel, rec_s - 1)
                cand3 = c3(cand)
                v_b, tr_b = bass.broadcast_tensor_aps(
                    v[:].rearrange("x (a m) -> x a m", a=1),
                    tr[:].rearrange("x (n p) -> x n p", n=K),
                )
                vector.tensor_tensor(out=cand3, in0=v_b, in1=tr_b, op=Alu.add)
                vector.drain()
                r = vector.tensor_reduce(out=newv[:], in_=cand3, axis=X, op=Alu.max)
                if rec_s is not None:
                    vector.drain().then_inc(vsem, 1)
                else:
                    vector.drain()
                vector.tensor_tensor(out=v[:], in0=newv[:], in1=feat_slice, op=Alu.add)
                vector.drain()
                if rec_s is not None and rec_s >= 1:
                    rmin(rec_s - 1)   # previous step's argmin
                    vector.drain()

            for s in range(W):
                step(fw[:, s * K:(s + 1) * K], None)

            # blend: record start = max(v + m1, blend_tile); on core 0 lane 0
            # m1 = -2^17 and blend = init_v, forcing the true initial vector.
            vector.tensor_scalar_add(out=newv_a[:], in0=v[:], scalar1=m1t[:, 0:1])
            vector.drain()
            vector.tensor_tensor(out=v[:], in0=newv_a[:], in1=bl[:], op=Alu.max)
            vector.drain()

            for s in range(L):
                step(fr[:, s * K:(s + 1) * K], s)

            rmin(L - 1)
            vector.drain().then_inc(v_sem, 1)

        @block.gpsimd
        def _(gpsimd):
            from concourse import library_config
            gpsimd.load_library(library_config.standard)
            for s in range(L):
                i = s % 2
                cand, newv = cands[i], newvs[i]
                gpsimd.wait_ge(vsem, s + 1)
                # d[n, p] = newv[n] - cand[n, p]  (exactly 0.0 at the argmax set)
                nv_b, cand_b_ = bass.broadcast_tensor_aps(
                    newv[:].rearrange("x (m a) -> x m a", a=1), c3(cand)
                )
                gpsimd.tensor_tensor(
                    out=c3(dg), in0=nv_b, in1=cand_b_, op=Alu.subtract,
                )
                gpsimd.drain().then_inc(grel, 1)
                # key = d + p*EPS: the tiny index term survives only where d == 0
                dg_b, io_b = bass.broadcast_tensor_aps(
                    c3(dg), io[:].rearrange("x (a p) -> x a p", a=1)
                )
                gpsimd.tensor_tensor(
                    out=c3(keygs[i]), in0=dg_b, in1=io_b, op=Alu.add,
                )
                gpsimd.drain().then_inc(gkey, 1)

    nc.compile()
    return nc


def _host_inputs(feats, trans):
    """Build per-core input dicts. feats [T,K] f32, trans [K,K] f32."""
    feats = np.ascontiguousarray(feats, dtype=np.float32)
    trans = np.ascontiguousarray(trans, dtype=np.float32)

    # exact fp32 prefix (reference arithmetic) for warm starts + drift rate
    v = np.full(K, NEG_INF, dtype=np.float32)
    v[START] = np.float32(0.0)
    vs = np.empty((P0 + 1, K), np.float32)
    vs[0] = v
    for t in range(P0):
        v = (v[None, :] + trans).max(axis=1) + feats[t]
        vs[t + 1] = v
    lam = float(vs[P0].max()) / P0

    ws = np.zeros((C, K), np.float32)
    for c in range(C):
        t0c = L * c - W
        if t0c < 0:
            ws[c] = 0.0
        elif t0c <= P0:
            ws[c] = vs[t0c]
        else:
            ws[c] = np.float32(lam * t0c)

    F = np.concatenate([np.zeros((W, K), np.float32), feats], axis=0)
    # lane c: warmup rows F[L*c : L*c+W], record rows F[L*c+W : L*c+W+L]
    sl = np.lib.stride_tricks.sliding_window_view(F, (W + L, K), axis=(0, 1))
    lanes = sl[np.arange(C) * L, 0]            # [C, W+L, K]
    featw_all = lanes[:, :W].reshape(C, W * K)
    featr_all = lanes[:, W:].reshape(C, L * K)

    init_v = np.full(K, NEG_INF, np.float32)
    init_v[START] = np.float32(0.0)

    transrep = np.broadcast_to(trans.reshape(1, K * K), (LPC, K * K))
    # packed index row: value at col p is p * EPS (broadcast over n on device)
    iotapm = np.broadcast_to(
        (_EPS * np.arange(K, dtype=np.float32)).reshape(1, K), (LPC, K)
    )

    in_maps = []
    for k in range(NCORES):
        lanes_k = slice(k * LPC, (k + 1) * LPC)
        blend_k = np.full((LPC, K), np.float32(-65536.0))
        m1_k = np.zeros((LPC, 1), np.float32)
        if k == 0:
            blend_k[0] = init_v
            m1_k[0] = np.float32(-131072.0)
        in_maps.append({
            "warm": np.ascontiguousarray(ws[lanes_k]),
            "featw": np.ascontiguousarray(featw_all[lanes_k]),
            "featr": np.ascontiguousarray(featr_all[lanes_k]),
            "transrep": np.ascontiguousarray(transrep),
            "iotapm": np.ascontiguousarray(iotapm),
            "blend": blend_k,
            "m1": m1_k,
        })
    return in_maps


def _postprocess(feats, trans, bp_f32, vend_last):
    """bp_f32 [T, K] stored as index * EPS; vend_last [K] final lane state."""
    bp = np.rint(bp_f32.astype(np.float64) * float(2.0 ** 120)).astype(np.int32)
    terminal = vend_last + trans[STOP]
    best_last = int(np.argmax(terminal))

    # vectorized backtrack: compose within 16-step chunks, then chain chunks
    bpc = bp.reshape(C, L, K)
    comp = np.broadcast_to(np.arange(K, dtype=np.int32), (C, K)).copy()
    lane_i = np.arange(C)
    for s in range(L - 1, -1, -1):
        comp = bpc[lane_i[:, None], s, comp]   # comp[c, e] = state before step s
    # entry[c] = state at time 16c - 1 given exit state at time 16c+15
    exits = np.empty(C, np.int32)
    e = best_last
    for c in range(C - 1, -1, -1):
        exits[c] = e
        e = comp[c, e]
    path = np.empty((C, L), np.int32)
    cur = exits
    for s in range(L - 1, -1, -1):
        path[:, s] = cur
        cur = bpc[lane_i, s, cur]
    path = path.reshape(T)

    # reproduce the reference's fp32 left-fold score bitwise along the path
    prev = np.concatenate([[START], path[:-1]])
    tstep = trans[path, prev]                  # f32 [T]
    fstep = feats[np.arange(T), path]          # f32 [T]
    acc = np.float32(0.0)
    f32 = np.float32
    for a, b in zip(tstep.tolist(), fstep.tolist()):
        acc = f32(f32(acc + f32(a)) + f32(b))
    score = f32(acc + trans[STOP, path[-1]])
    return score, path


_last_results = None  # BassKernelResults of the most recent run (for profiling)


def kernel(feats, transitions):
    global _last_results
    feats = np.ascontiguousarray(feats, dtype=np.float32)
    trans = np.ascontiguousarray(transitions, dtype=np.float32)

    from concourse.bass_utils import run_bass_kernel_spmd

    nc = _build_nc()
    in_maps = _host_inputs(feats, trans)
    res = run_bass_kernel_spmd(nc, in_maps, list(range(NCORES)))
    _last_results = res

    bp_f32 = np.empty((T, K), np.float32)
    for k in range(NCORES):
        bp_k = np.asarray(res.results[k]["bp"]).reshape(LPC, L, K)
        t0k = 16 * (k * LPC)
        bp_f32[t0k:t0k + LPC * L] = bp_k.reshape(LPC * L, K)
    vend_last = np.asarray(res.results[NCORES - 1]["vend"])[LPC - 1]

    score, path = _postprocess(feats, trans, bp_f32, vend_last)
    return score, path.astype(np.int32)


# revision 30
# speedup vs baseline: 1.4457x; 1.0299x over previous
"""Viterbi (CRF decode) kernel for Trainium2, 8 NeuronCores.

Problem: single sequence T=16384, K=35 tags. reference computes a forward
Viterbi pass (max-plus recurrence over time, sequential), backpointers, then
backtracks; returns (path_score, best_path[T]).

Strategy: the time recurrence is chunked into 1024 lanes of 16 steps
(128 lanes per core x 8 cores, lane = one 16-step record window). Each lane
first runs a 32-step warmup from a magnitude-matched warm-start vector so
that, by tropical (max-plus) coalescence, its state equals the true forward
state up to a uniform additive constant that is a multiple of the local
float32 ulp grid -- which makes every subsequent fp32 add/max/argmax
bitwise-identical to the reference's (shift by a grid multiple commutes with
rounding within a binade). The record phase then emits exact backpointers
for its 16 time steps. Host side: a short exact prefix (512 steps) supplies
true warm starts for early lanes plus a drift estimate for the magnitude
guesses, and afterwards backtracks the backpointers and reproduces the
reference's fp32-accumulated path score bitwise by folding along the path.
"""

import numpy as np

K = 35
START = 33
STOP = 34
T = 16384
NEG_INF = np.float32(-10000.0)

W = 24          # warmup steps per lane
L = 16          # recorded steps per lane
P0 = 512        # host-exact prefix length
NCORES = 8
LPC = 128       # lanes per core
C = T // L      # 1024 lanes total

_BIG = float(2.0 ** 30)
_EPS = np.float32(2.0 ** -120)  # index packing scale: key = d + p*EPS


def _build_nc():
    import concourse.bass as bass
    import concourse.bacc as bacc
    import concourse.mybir as mybir

    f32 = mybir.dt.float32
    Alu = mybir.AluOpType
    X = mybir.AxisListType.X

    nc = bacc.Bacc(detect_race_conditions=False)

    warm = nc.declare_dram_parameter("warm", [LPC, K], f32, isOutput=False)
    featw = nc.declare_dram_parameter("featw", [LPC, W * K], f32, isOutput=False)
    featr = nc.declare_dram_parameter("featr", [LPC, L * K], f32, isOutput=False)
    transrep = nc.declare_dram_parameter("transrep", [LPC, K * K], f32, isOutput=False)
    iotapm = nc.declare_dram_parameter("iotapm", [LPC, K], f32, isOutput=False)
    blend = nc.declare_dram_parameter("blend", [LPC, K], f32, isOutput=False)
    m1 = nc.declare_dram_parameter("m1", [LPC, 1], f32, isOutput=False)
    bp_out = nc.declare_dram_parameter("bp", [LPC, L * K], f32, isOutput=True)
    vend_out = nc.declare_dram_parameter("vend", [LPC, K], f32, isOutput=True)

    XV = 10  # bp columns n < XV on Vector; n >= XV on GpSimd

    from contextlib import ExitStack

    with ExitStack() as ctx:
        tr = ctx.enter_context(nc.sbuf_tensor([LPC, K * K], f32))
        io = ctx.enter_context(nc.sbuf_tensor([LPC, K], f32))
        fw = ctx.enter_context(nc.sbuf_tensor([LPC, W * K], f32))
        fr = ctx.enter_context(nc.sbuf_tensor([LPC, L * K], f32))
        bl = ctx.enter_context(nc.sbuf_tensor([LPC, K], f32))
        m1t = ctx.enter_context(nc.sbuf_tensor([LPC, 1], f32))
        v = ctx.enter_context(nc.sbuf_tensor([LPC, K], f32))
        newv_a = ctx.enter_context(nc.sbuf_tensor([LPC, K], f32))
        newv_b = ctx.enter_context(nc.sbuf_tensor([LPC, K], f32))
        cand_a = ctx.enter_context(nc.sbuf_tensor([LPC, K * K], f32))
        cand_b = ctx.enter_context(nc.sbuf_tensor([LPC, K * K], f32))
        dg = ctx.enter_context(nc.sbuf_tensor([LPC, K * K], f32))
        keyg_a = ctx.enter_context(nc.sbuf_tensor([LPC, K * K], f32))
        keyg_b = ctx.enter_context(nc.sbuf_tensor([LPC, K * K], f32))
        bpt = ctx.enter_context(nc.sbuf_tensor([LPC, L * K], f32))
        dma_sem = ctx.enter_context(nc.semaphore())
        v_sem = ctx.enter_context(nc.semaphore())
        vsem = ctx.enter_context(nc.semaphore())   # vector -> gpsimd: step ready
        grel = ctx.enter_context(nc.semaphore())   # gpsimd -> vector: cand/newv consumed
        gkey = ctx.enter_context(nc.semaphore())   # gpsimd -> vector: key ready
        block = ctx.enter_context(nc.Block())
        cands = [cand_a, cand_b]
        newvs = [newv_a, newv_b]
        keygs = [keyg_a, keyg_b]

        def c3(t, lo=0, hi=K):
            return t[:, lo * K:hi * K].rearrange("x (n p) -> x n p", n=hi - lo)

        @block.sync
        def _(sync):
            for dst, src in (
                (tr, transrep), (io, iotapm), (fw, featw), (fr, featr),
                (bl, blend), (m1t, m1), (v, warm),
            ):
                sync.dma_start(out=dst[:], in_=src[:]).then_inc(dma_sem, 16)
            sync.wait_ge(v_sem, 1)
            sync.dma_start(out=bp_out[:], in_=bpt[:]).then_inc(dma_sem, 16)
            sync.dma_start(out=vend_out[:], in_=v[:]).then_inc(dma_sem, 16)

        @block.vector
        def _(vector):
            vector.wait_ge(dma_sem, 7 * 16)

            def rmin(rec_s):
                # stored bp[rec_s] = min over p of (d + p*EPS) = first_argmax * EPS
                vector.wait_ge(gkey, rec_s + 1)
                vector.tensor_reduce(
                    out=bpt[:, rec_s * K:(rec_s + 1) * K],
                    in_=c3(keygs[rec_s % 2]),
                    axis=X, op=Alu.min,
                )

            def step(feat_slice, rec_s):
                i = (rec_s or 0) % 2
                cand, newv = (cands[i], newvs[i]) if rec_s is not None else (cands[0], newvs[0])
                if rec_s is not None and rec_s >= 2:
                    # gpsimd must be done reading cand/newv of step rec_s-2
                    vector.wait_ge(grel, rec_s - 1)
                cand3 = c3(cand)
                v_b, tr_b = bass.broadcast_tensor_aps(
                    v[:].rearrange("x (a m) -> x a m", a=1),
                    tr[:].rearrange("x (n p) -> x n p", n=K),
                )
                vector.tensor_tensor(out=cand3, in0=v_b, in1=tr_b, op=Alu.add)
                r = vector.tensor_reduce(out=newv[:], in_=cand3, axis=X, op=Alu.max)
                if rec_s is not None:
                    vector.drain().then_inc(vsem, 1)
                else:
                    vector.drain()
                vector.tensor_tensor(out=v[:], in0=newv[:], in1=feat_slice, op=Alu.add)
                if rec_s is not None and rec_s >= 1:
                    rmin(rec_s - 1)   # previous step's argmin
                vector.drain()

            for s in range(W):
                step(fw[:, s * K:(s + 1) * K], None)

            # blend: record start = max(v + m1, blend_tile); on core 0 lane 0
            # m1 = -2^17 and blend = init_v, forcing the true initial vector.
            vector.tensor_scalar_add(out=newv_a[:], in0=v[:], scalar1=m1t[:, 0:1])
            vector.drain()
            vector.tensor_tensor(out=v[:], in0=newv_a[:], in1=bl[:], op=Alu.max)
            vector.drain()

            for s in range(L):
                step(fr[:, s * K:(s + 1) * K], s)

            rmin(L - 1)
            vector.drain().then_inc(v_sem, 1)

        @block.gpsimd
        def _(gpsimd):
            from concourse import library_config
            gpsimd.load_library(library_config.standard)
            for s in range(L):
                i = s % 2
                cand, newv = cands[i], newvs[i]
                gpsimd.wait_ge(vsem, s + 1)
                # d[n, p] = newv[n] - cand[n, p]  (exactly 0.0 at the argmax set)
                nv_b, cand_b_ = bass.broadcast_tensor_aps(
                    newv[:].rearrange("x (m a) -> x m a", a=1), c3(cand)
                )
                gpsimd.tensor_tensor(
                    out=c3(dg), in0=nv_b, in1=cand_b_, op=Alu.subtract,
                )
                # key = d + p*EPS: the tiny index term survives only where d == 0
                dg_b, io_b = bass.broadcast_tensor_aps(
                    c3(dg), io[:].rearrange("x (a p) -> x a p", a=1)
                )
                gpsimd.tensor_tensor(
                    out=c3(keygs[i]), in0=dg_b, in1=io_b, op=Alu.add,
                )
                gpsimd.drain().then_inc(gkey, 1)
                gpsimd.engine_nop().then_inc(grel, 1)

    nc.compile()
    return nc


def _host_inputs(feats, trans):
    """Build per-core input dicts. feats [T,K] f32, trans [K,K] f32."""
    feats = np.ascontiguousarray(feats, dtype=np.float32)
    trans = np.ascontiguousarray(trans, dtype=np.float32)

    # exact fp32 prefix (reference arithmetic) for warm starts + drift rate
    v = np.full(K, NEG_INF, dtype=np.float32)
    v[START] = np.float32(0.0)
    vs = np.empty((P0 + 1, K), np.float32)
    vs[0] = v
    for t in range(P0):
        v = (v[None, :] + trans).max(axis=1) + feats[t]
        vs[t + 1] = v
    lam = float(vs[P0].max()) / P0

    ws = np.zeros((C, K), np.float32)
    for c in range(C):
        t0c = L * c - W
        if t0c < 0:
            ws[c] = 0.0
        elif t0c <= P0:
            ws[c] = vs[t0c]
        else:
            ws[c] = np.float32(lam * t0c)

    F = np.concatenate([np.zeros((W, K), np.float32), feats], axis=0)
    # lane c: warmup rows F[L*c : L*c+W], record rows F[L*c+W : L*c+W+L]
    sl = np.lib.stride_tricks.sliding_window_view(F, (W + L, K), axis=(0, 1))
    lanes = sl[np.arange(C) * L, 0]            # [C, W+L, K]
    featw_all = lanes[:, :W].reshape(C, W * K)
    featr_all = lanes[:, W:].reshape(C, L * K)

    init_v = np.full(K, NEG_INF, np.float32)
    init_v[START] = np.float32(0.0)

    transrep = np.broadcast_to(trans.reshape(1, K * K), (LPC, K * K))
    # packed index row: value at col p is p * EPS (broadcast over n on device)
    iotapm = np.broadcast_to(
        (_EPS * np.arange(K, dtype=np.float32)).reshape(1, K), (LPC, K)
    )

    in_maps = []
    for k in range(NCORES):
        lanes_k = slice(k * LPC, (k + 1) * LPC)
        blend_k = np.full((LPC, K), np.float32(-65536.0))
        m1_k = np.zeros((LPC, 1), np.float32)
        if k == 0:
            blend_k[0] = init_v
            m1_k[0] = np.float32(-131072.0)
        in_maps.append({
            "warm": np.ascontiguousarray(ws[lanes_k]),
            "featw": np.ascontiguousarray(featw_all[lanes_k]),
            "featr": np.ascontiguousarray(featr_all[lanes_k]),
            "transrep": np.ascontiguousarray(transrep),
            "iotapm": np.ascontiguousarray(iotapm),
            "blend": blend_k,
            "m1": m1_k,
        })
    return in_maps


def _postprocess(feats, trans, bp_f32, vend_last):
    """bp_f32 [T, K] stored as index * EPS; vend_last [K] final lane state."""
    bp = np.rint(bp_f32.astype(np.float64) * float(2.0 ** 120)).astype(np.int32)
    terminal = vend_last + trans[STOP]
    best_last = int(np.argmax(terminal))

    # vectorized backtrack: compose within 16-step chunks, then chain chunks
    bpc = bp.reshape(C, L, K)
    comp = np.broadcast_to(np.arange(K, dtype=np.int32), (C, K)).copy()
    lane_i = np.arange(C)
    for s in range(L - 1, -1, -1):
        comp = bpc[lane_i[:, None], s, comp]   # comp[c, e] = state before step s
    # entry[c] = state at time 16c - 1 given exit state at time 16c+15
    exits = np.empty(C, np.int32)
    e = best_last
    for c in range(C - 1, -1, -1):
        exits[c] = e
        e = comp[c, e]
    path = np.empty((C, L), np.int32)
    cur = exits
    for s in range(L - 1, -1, -1):
        path[:, s] = cur
        cur = bpc[lane_i, s, cur]
    path = path.reshape(T)

    # reproduce the reference's fp32 left-fold score bitwise along the path
    prev = np.concatenate([[START], path[:-1]])
    tstep = trans[path, prev]                  # f32 [T]
    fstep = feats[np.arange(T), path]          # f32 [T]
    acc = np.float32(0.0)
    f32 = np.float32
    for a, b in zip(tstep.tolist(), fstep.tolist()):
        acc = f32(f32(acc + f32(a)) + f32(b))
    score = f32(acc + trans[STOP, path[-1]])
    return score, path


_last_results = None  # BassKernelResults of the most recent run (for profiling)


def kernel(feats, transitions):
    global _last_results
    feats = np.ascontiguousarray(feats, dtype=np.float32)
    trans = np.ascontiguousarray(transitions, dtype=np.float32)

    from concourse.bass_utils import run_bass_kernel_spmd

    nc = _build_nc()
    in_maps = _host_inputs(feats, trans)
    res = run_bass_kernel_spmd(nc, in_maps, list(range(NCORES)))
    _last_results = res

    bp_f32 = np.empty((T, K), np.float32)
    for k in range(NCORES):
        bp_k = np.asarray(res.results[k]["bp"]).reshape(LPC, L, K)
        t0k = 16 * (k * LPC)
        bp_f32[t0k:t0k + LPC * L] = bp_k.reshape(LPC * L, K)
    vend_last = np.asarray(res.results[NCORES - 1]["vend"])[LPC - 1]

    score, path = _postprocess(feats, trans, bp_f32, vend_last)
    return score, path.astype(np.int32)


# revision 31
# speedup vs baseline: 1.4944x; 1.0337x over previous
"""Viterbi (CRF decode) kernel for Trainium2, 8 NeuronCores.

Problem: single sequence T=16384, K=35 tags. reference computes a forward
Viterbi pass (max-plus recurrence over time, sequential), backpointers, then
backtracks; returns (path_score, best_path[T]).

Strategy: the time recurrence is chunked into 1024 lanes of 16 steps
(128 lanes per core x 8 cores, lane = one 16-step record window). Each lane
first runs a 32-step warmup from a magnitude-matched warm-start vector so
that, by tropical (max-plus) coalescence, its state equals the true forward
state up to a uniform additive constant that is a multiple of the local
float32 ulp grid -- which makes every subsequent fp32 add/max/argmax
bitwise-identical to the reference's (shift by a grid multiple commutes with
rounding within a binade). The record phase then emits exact backpointers
for its 16 time steps. Host side: a short exact prefix (512 steps) supplies
true warm starts for early lanes plus a drift estimate for the magnitude
guesses, and afterwards backtracks the backpointers and reproduces the
reference's fp32-accumulated path score bitwise by folding along the path.
"""

import numpy as np

K = 35
START = 33
STOP = 34
T = 16384
NEG_INF = np.float32(-10000.0)

W = 24          # warmup steps per lane
L = 16          # recorded steps per lane
P0 = 512        # host-exact prefix length
NCORES = 8
LPC = 128       # lanes per core
C = T // L      # 1024 lanes total

_BIG = float(2.0 ** 30)
_EPS = np.float32(2.0 ** -120)  # index packing scale: key = d + p*EPS


def _build_nc():
    import concourse.bass as bass
    import concourse.bacc as bacc
    import concourse.mybir as mybir

    f32 = mybir.dt.float32
    Alu = mybir.AluOpType
    X = mybir.AxisListType.X

    nc = bacc.Bacc(detect_race_conditions=False)

    warm = nc.declare_dram_parameter("warm", [LPC, K], f32, isOutput=False)
    featw = nc.declare_dram_parameter("featw", [LPC, W * K], f32, isOutput=False)
    featr = nc.declare_dram_parameter("featr", [LPC, L * K], f32, isOutput=False)
    transrep = nc.declare_dram_parameter("transrep", [LPC, K * K], f32, isOutput=False)
    iotapm = nc.declare_dram_parameter("iotapm", [LPC, K], f32, isOutput=False)
    blend = nc.declare_dram_parameter("blend", [LPC, K], f32, isOutput=False)
    m1 = nc.declare_dram_parameter("m1", [LPC, 1], f32, isOutput=False)
    bp_out = nc.declare_dram_parameter("bp", [LPC, L * K], f32, isOutput=True)
    vend_out = nc.declare_dram_parameter("vend", [LPC, K], f32, isOutput=True)

    XV = 10  # bp columns n < XV on Vector; n >= XV on GpSimd

    from contextlib import ExitStack

    with ExitStack() as ctx:
        tr = ctx.enter_context(nc.sbuf_tensor([LPC, K * K], f32))
        io = ctx.enter_context(nc.sbuf_tensor([LPC, K], f32))
        fw = ctx.enter_context(nc.sbuf_tensor([LPC, W * K], f32))
        fr = ctx.enter_context(nc.sbuf_tensor([LPC, L * K], f32))
        bl = ctx.enter_context(nc.sbuf_tensor([LPC, K], f32))
        m1t = ctx.enter_context(nc.sbuf_tensor([LPC, 1], f32))
        v = ctx.enter_context(nc.sbuf_tensor([LPC, K], f32))
        newv_a = ctx.enter_context(nc.sbuf_tensor([LPC, K], f32))
        newv_b = ctx.enter_context(nc.sbuf_tensor([LPC, K], f32))
        cand_a = ctx.enter_context(nc.sbuf_tensor([LPC, K * K], f32))
        cand_b = ctx.enter_context(nc.sbuf_tensor([LPC, K * K], f32))
        dg = ctx.enter_context(nc.sbuf_tensor([LPC, K * K], f32))
        keyg_a = ctx.enter_context(nc.sbuf_tensor([LPC, K * K], f32))
        keyg_b = ctx.enter_context(nc.sbuf_tensor([LPC, K * K], f32))
        bpt = ctx.enter_context(nc.sbuf_tensor([LPC, L * K], f32))
        dma_sem = ctx.enter_context(nc.semaphore())
        v_sem = ctx.enter_context(nc.semaphore())
        vsem = ctx.enter_context(nc.semaphore())   # vector -> gpsimd: step ready
        grel = ctx.enter_context(nc.semaphore())   # gpsimd -> vector: cand/newv consumed
        gkey = ctx.enter_context(nc.semaphore())   # gpsimd -> vector: key ready
        block = ctx.enter_context(nc.Block())
        cands = [cand_a, cand_b]
        newvs = [newv_a, newv_b]
        keygs = [keyg_a, keyg_b]

        def c3(t, lo=0, hi=K):
            return t[:, lo * K:hi * K].rearrange("x (n p) -> x n p", n=hi - lo)

        @block.sync
        def _(sync):
            for dst, src in (
                (v, warm), (tr, transrep), (fw, featw),
                (fr, featr), (bl, blend), (m1t, m1), (io, iotapm),
            ):
                sync.dma_start(out=dst[:], in_=src[:]).then_inc(dma_sem, 16)
            sync.wait_ge(v_sem, 1)
            sync.dma_start(out=bp_out[:], in_=bpt[:]).then_inc(dma_sem, 16)
            sync.dma_start(out=vend_out[:], in_=v[:]).then_inc(dma_sem, 16)

        @block.vector
        def _(vector):
            vector.wait_ge(dma_sem, 3 * 16)

            def rmin(rec_s):
                # stored bp[rec_s] = min over p of (d + p*EPS) = first_argmax * EPS
                vector.wait_ge(gkey, rec_s + 1)
                vector.tensor_reduce(
                    out=bpt[:, rec_s * K:(rec_s + 1) * K],
                    in_=c3(keygs[rec_s % 2]),
                    axis=X, op=Alu.min,
                )

            def step(feat_slice, rec_s):
                i = (rec_s or 0) % 2
                cand, newv = (cands[i], newvs[i]) if rec_s is not None else (cands[0], newvs[0])
                if rec_s is not None and rec_s >= 2:
                    # gpsimd must be done reading cand/newv of step rec_s-2
                    vector.wait_ge(grel, rec_s - 1)
                cand3 = c3(cand)
                v_b, tr_b = bass.broadcast_tensor_aps(
                    v[:].rearrange("x (a m) -> x a m", a=1),
                    tr[:].rearrange("x (n p) -> x n p", n=K),
                )
                vector.tensor_tensor(out=cand3, in0=v_b, in1=tr_b, op=Alu.add)
                r = vector.tensor_reduce(out=newv[:], in_=cand3, axis=X, op=Alu.max)
                if rec_s is not None:
                    vector.drain().then_inc(vsem, 1)
                vector.tensor_tensor(out=v[:], in0=newv[:], in1=feat_slice, op=Alu.add)
                if rec_s is not None and rec_s >= 1:
                    rmin(rec_s - 1)   # previous step's argmin
                vector.drain()

            for s in range(W):
                step(fw[:, s * K:(s + 1) * K], None)

            # blend: record start = max(v + m1, blend_tile); on core 0 lane 0
            # m1 = -2^17 and blend = init_v, forcing the true initial vector.
            vector.wait_ge(dma_sem, 7 * 16)
            vector.tensor_scalar_add(out=newv_a[:], in0=v[:], scalar1=m1t[:, 0:1])
            vector.drain()
            vector.tensor_tensor(out=v[:], in0=newv_a[:], in1=bl[:], op=Alu.max)
            vector.drain()

            for s in range(L):
                step(fr[:, s * K:(s + 1) * K], s)

            rmin(L - 1)
            vector.drain().then_inc(v_sem, 1)

        @block.gpsimd
        def _(gpsimd):
            from concourse import library_config
            gpsimd.load_library(library_config.standard)
            for s in range(L):
                i = s % 2
                cand, newv = cands[i], newvs[i]
                gpsimd.wait_ge(vsem, s + 1)
                # d[n, p] = newv[n] - cand[n, p]  (exactly 0.0 at the argmax set)
                nv_b, cand_b_ = bass.broadcast_tensor_aps(
                    newv[:].rearrange("x (m a) -> x m a", a=1), c3(cand)
                )
                gpsimd.tensor_tensor(
                    out=c3(dg), in0=nv_b, in1=cand_b_, op=Alu.subtract,
                )
                # key = d + p*EPS: the tiny index term survives only where d == 0
                dg_b, io_b = bass.broadcast_tensor_aps(
                    c3(dg), io[:].rearrange("x (a p) -> x a p", a=1)
                )
                gpsimd.tensor_tensor(
                    out=c3(keygs[i]), in0=io_b, in1=dg_b, op=Alu.add,
                )
                gpsimd.drain().then_inc(gkey, 1)
                gpsimd.engine_nop().then_inc(grel, 1)

    nc.compile()
    return nc


def _host_inputs(feats, trans):
    """Build per-core input dicts. feats [T,K] f32, trans [K,K] f32."""
    feats = np.ascontiguousarray(feats, dtype=np.float32)
    trans = np.ascontiguousarray(trans, dtype=np.float32)

    # exact fp32 prefix (reference arithmetic) for warm starts + drift rate
    v = np.full(K, NEG_INF, dtype=np.float32)
    v[START] = np.float32(0.0)
    vs = np.empty((P0 + 1, K), np.float32)
    vs[0] = v
    for t in range(P0):
        v = (v[None, :] + trans).max(axis=1) + feats[t]
        vs[t + 1] = v
    lam = float(vs[P0].max()) / P0

    ws = np.zeros((C, K), np.float32)
    for c in range(C):
        t0c = L * c - W
        if t0c < 0:
            ws[c] = 0.0
        elif t0c <= P0:
            ws[c] = vs[t0c]
        else:
            ws[c] = np.float32(lam * t0c)

    F = np.concatenate([np.zeros((W, K), np.float32), feats], axis=0)
    # lane c: warmup rows F[L*c : L*c+W], record rows F[L*c+W : L*c+W+L]
    sl = np.lib.stride_tricks.sliding_window_view(F, (W + L, K), axis=(0, 1))
    lanes = sl[np.arange(C) * L, 0]            # [C, W+L, K]
    featw_all = lanes[:, :W].reshape(C, W * K)
    featr_all = lanes[:, W:].reshape(C, L * K)

    init_v = np.full(K, NEG_INF, np.float32)
    init_v[START] = np.float32(0.0)

    transrep = np.broadcast_to(trans.reshape(1, K * K), (LPC, K * K))
    # packed index row: value at col p is p * EPS (broadcast over n on device)
    iotapm = np.broadcast_to(
        (_EPS * np.arange(K, dtype=np.float32)).reshape(1, K), (LPC, K)
    )

    in_maps = []
    for k in range(NCORES):
        lanes_k = slice(k * LPC, (k + 1) * LPC)
        blend_k = np.full((LPC, K), np.float32(-65536.0))
        m1_k = np.zeros((LPC, 1), np.float32)
        if k == 0:
            blend_k[0] = init_v
            m1_k[0] = np.float32(-131072.0)
        in_maps.append({
            "warm": np.ascontiguousarray(ws[lanes_k]),
            "featw": np.ascontiguousarray(featw_all[lanes_k]),
            "featr": np.ascontiguousarray(featr_all[lanes_k]),
            "transrep": np.ascontiguousarray(transrep),
            "iotapm": np.ascontiguousarray(iotapm),
            "blend": blend_k,
            "m1": m1_k,
        })
    return in_maps


def _postprocess(feats, trans, bp_f32, vend_last):
    """bp_f32 [T, K] stored as index * EPS; vend_last [K] final lane state."""
    bp = np.rint(bp_f32.astype(np.float64) * float(2.0 ** 120)).astype(np.int32)
    terminal = vend_last + trans[STOP]
    best_last = int(np.argmax(terminal))

    # vectorized backtrack: compose within 16-step chunks, then chain chunks
    bpc = bp.reshape(C, L, K)
    comp = np.broadcast_to(np.arange(K, dtype=np.int32), (C, K)).copy()
    lane_i = np.arange(C)
    for s in range(L - 1, -1, -1):
        comp = bpc[lane_i[:, None], s, comp]   # comp[c, e] = state before step s
    # entry[c] = state at time 16c - 1 given exit state at time 16c+15
    exits = np.empty(C, np.int32)
    e = best_last
    for c in range(C - 1, -1, -1):
        exits[c] = e
        e = comp[c, e]
    path = np.empty((C, L), np.int32)
    cur = exits
    for s in range(L - 1, -1, -1):
        path[:, s] = cur
        cur = bpc[lane_i, s, cur]
    path = path.reshape(T)

    # reproduce the reference's fp32 left-fold score bitwise along the path
    prev = np.concatenate([[START], path[:-1]])
    tstep = trans[path, prev]                  # f32 [T]
    fstep = feats[np.arange(T), path]          # f32 [T]
    acc = np.float32(0.0)
    f32 = np.float32
    for a, b in zip(tstep.tolist(), fstep.tolist()):
        acc = f32(f32(acc + f32(a)) + f32(b))
    score = f32(acc + trans[STOP, path[-1]])
    return score, path


_last_results = None  # BassKernelResults of the most recent run (for profiling)


def kernel(feats, transitions):
    global _last_results
    feats = np.ascontiguousarray(feats, dtype=np.float32)
    trans = np.ascontiguousarray(transitions, dtype=np.float32)

    from concourse.bass_utils import run_bass_kernel_spmd

    nc = _build_nc()
    in_maps = _host_inputs(feats, trans)
    res = run_bass_kernel_spmd(nc, in_maps, list(range(NCORES)))
    _last_results = res

    bp_f32 = np.empty((T, K), np.float32)
    for k in range(NCORES):
        bp_k = np.asarray(res.results[k]["bp"]).reshape(LPC, L, K)
        t0k = 16 * (k * LPC)
        bp_f32[t0k:t0k + LPC * L] = bp_k.reshape(LPC * L, K)
    vend_last = np.asarray(res.results[NCORES - 1]["vend"])[LPC - 1]

    score, path = _postprocess(feats, trans, bp_f32, vend_last)
    return score, path.astype(np.int32)


# revision 32
# speedup vs baseline: 1.5273x; 1.0220x over previous
"""Viterbi (CRF decode) kernel for Trainium2, 8 NeuronCores.

Problem: single sequence T=16384, K=35 tags. reference computes a forward
Viterbi pass (max-plus recurrence over time, sequential), backpointers, then
backtracks; returns (path_score, best_path[T]).

Strategy: the time recurrence is chunked into 1024 lanes of 16 steps
(128 lanes per core x 8 cores, lane = one 16-step record window). Each lane
first runs a 32-step warmup from a magnitude-matched warm-start vector so
that, by tropical (max-plus) coalescence, its state equals the true forward
state up to a uniform additive constant that is a multiple of the local
float32 ulp grid -- which makes every subsequent fp32 add/max/argmax
bitwise-identical to the reference's (shift by a grid multiple commutes with
rounding within a binade). The record phase then emits exact backpointers
for its 16 time steps. Host side: a short exact prefix (512 steps) supplies
true warm starts for early lanes plus a drift estimate for the magnitude
guesses, and afterwards backtracks the backpointers and reproduces the
reference's fp32-accumulated path score bitwise by folding along the path.
"""

import numpy as np

K = 35
START = 33
STOP = 34
T = 16384
NEG_INF = np.float32(-10000.0)

W = 24          # warmup steps per lane
L = 16          # recorded steps per lane
P0 = 512        # host-exact prefix length
NCORES = 8
LPC = 128       # lanes per core
C = T // L      # 1024 lanes total

_BIG = float(2.0 ** 30)
_EPS = np.float32(2.0 ** -120)  # index packing scale: key = d + p*EPS


def _build_nc():
    import concourse.bass as bass
    import concourse.bacc as bacc
    import concourse.mybir as mybir

    f32 = mybir.dt.float32
    bf16 = mybir.dt.bfloat16
    Alu = mybir.AluOpType
    X = mybir.AxisListType.X

    nc = bacc.Bacc(detect_race_conditions=False)

    warm = nc.declare_dram_parameter("warm", [LPC, K], f32, isOutput=False)
    featw = nc.declare_dram_parameter("featw", [LPC, W * K], f32, isOutput=False)
    featr = nc.declare_dram_parameter("featr", [LPC, L * K], f32, isOutput=False)
    transrep = nc.declare_dram_parameter("transrep", [LPC, K * K], f32, isOutput=False)
    iotapm = nc.declare_dram_parameter("iotapm", [LPC, K], bf16, isOutput=False)
    blend = nc.declare_dram_parameter("blend", [LPC, K], f32, isOutput=False)
    m1 = nc.declare_dram_parameter("m1", [LPC, 1], f32, isOutput=False)
    bp_out = nc.declare_dram_parameter("bp", [LPC, L * K], f32, isOutput=True)
    vend_out = nc.declare_dram_parameter("vend", [LPC, K], f32, isOutput=True)

    XV = 10  # bp columns n < XV on Vector; n >= XV on GpSimd

    from contextlib import ExitStack

    with ExitStack() as ctx:
        tr = ctx.enter_context(nc.sbuf_tensor([LPC, K * K], f32))
        io = ctx.enter_context(nc.sbuf_tensor([LPC, K], bf16))
        fw = ctx.enter_context(nc.sbuf_tensor([LPC, W * K], f32))
        fr = ctx.enter_context(nc.sbuf_tensor([LPC, L * K], f32))
        bl = ctx.enter_context(nc.sbuf_tensor([LPC, K], f32))
        m1t = ctx.enter_context(nc.sbuf_tensor([LPC, 1], f32))
        v = ctx.enter_context(nc.sbuf_tensor([LPC, K], f32))
        newv_a = ctx.enter_context(nc.sbuf_tensor([LPC, K], f32))
        newv_b = ctx.enter_context(nc.sbuf_tensor([LPC, K], f32))
        cand_a = ctx.enter_context(nc.sbuf_tensor([LPC, K * K], f32))
        cand_b = ctx.enter_context(nc.sbuf_tensor([LPC, K * K], f32))
        dg = ctx.enter_context(nc.sbuf_tensor([LPC, K * K], bf16))
        keyg_a = ctx.enter_context(nc.sbuf_tensor([LPC, K * K], bf16))
        keyg_b = ctx.enter_context(nc.sbuf_tensor([LPC, K * K], bf16))
        bpt = ctx.enter_context(nc.sbuf_tensor([LPC, L * K], f32))
        dma_sem = ctx.enter_context(nc.semaphore())
        v_sem = ctx.enter_context(nc.semaphore())
        vsem = ctx.enter_context(nc.semaphore())   # vector -> gpsimd: step ready
        grel = ctx.enter_context(nc.semaphore())   # gpsimd -> vector: cand/newv consumed
        gkey = ctx.enter_context(nc.semaphore())   # gpsimd -> vector: key ready
        block = ctx.enter_context(nc.Block())
        cands = [cand_a, cand_b]
        newvs = [newv_a, newv_b]
        keygs = [keyg_a, keyg_b]

        def c3(t, lo=0, hi=K):
            return t[:, lo * K:hi * K].rearrange("x (n p) -> x n p", n=hi - lo)

        @block.sync
        def _(sync):
            for dst, src in (
                (v, warm), (tr, transrep), (fw, featw),
                (fr, featr), (bl, blend), (m1t, m1), (io, iotapm),
            ):
                sync.dma_start(out=dst[:], in_=src[:]).then_inc(dma_sem, 16)
            sync.wait_ge(v_sem, 1)
            sync.dma_start(out=bp_out[:], in_=bpt[:]).then_inc(dma_sem, 16)
            sync.dma_start(out=vend_out[:], in_=v[:]).then_inc(dma_sem, 16)

        @block.vector
        def _(vector):
            vector.wait_ge(dma_sem, 3 * 16)

            def rmin(rec_s):
                # stored bp[rec_s] = min over p of (d + p*EPS) = first_argmax * EPS
                vector.wait_ge(gkey, rec_s + 1)
                vector.tensor_reduce(
                    out=bpt[:, rec_s * K:(rec_s + 1) * K],
                    in_=c3(keygs[rec_s % 2]),
                    axis=X, op=Alu.min,
                )

            def step(feat_slice, rec_s):
                i = (rec_s or 0) % 2
                cand, newv = (cands[i], newvs[i]) if rec_s is not None else (cands[0], newvs[0])
                if rec_s is not None and rec_s >= 2:
                    # gpsimd must be done reading cand/newv of step rec_s-2
                    vector.wait_ge(grel, rec_s - 1)
                cand3 = c3(cand)
                v_b, tr_b = bass.broadcast_tensor_aps(
                    v[:].rearrange("x (a m) -> x a m", a=1),
                    tr[:].rearrange("x (n p) -> x n p", n=K),
                )
                vector.tensor_tensor(out=cand3, in0=v_b, in1=tr_b, op=Alu.add)
                r = vector.tensor_reduce(out=newv[:], in_=cand3, axis=X, op=Alu.max)
                if rec_s is not None:
                    vector.drain().then_inc(vsem, 1)
                vector.tensor_tensor(out=v[:], in0=newv[:], in1=feat_slice, op=Alu.add)
                if rec_s is not None and rec_s >= 1:
                    rmin(rec_s - 1)   # previous step's argmin
                vector.drain()

            for s in range(W):
                step(fw[:, s * K:(s + 1) * K], None)

            # blend: record start = max(v + m1, blend_tile); on core 0 lane 0
            # m1 = -2^17 and blend = init_v, forcing the true initial vector.
            vector.wait_ge(dma_sem, 7 * 16)
            vector.tensor_scalar_add(out=newv_a[:], in0=v[:], scalar1=m1t[:, 0:1])
            vector.drain()
            vector.tensor_tensor(out=v[:], in0=newv_a[:], in1=bl[:], op=Alu.max)
            vector.drain()

            for s in range(L):
                step(fr[:, s * K:(s + 1) * K], s)

            rmin(L - 1)
            vector.drain().then_inc(v_sem, 1)

        @block.gpsimd
        def _(gpsimd):
            from concourse import library_config
            gpsimd.load_library(library_config.standard)
            for s in range(L):
                i = s % 2
                cand, newv = cands[i], newvs[i]
                gpsimd.wait_ge(vsem, s + 1)
                # d[n, p] = newv[n] - cand[n, p]  (exactly 0.0 at the argmax set)
                nv_b, cand_b_ = bass.broadcast_tensor_aps(
                    newv[:].rearrange("x (m a) -> x m a", a=1), c3(cand)
                )
                gpsimd.tensor_tensor(
                    out=c3(dg), in0=nv_b, in1=cand_b_, op=Alu.subtract,
                )
                # key = d + p*EPS: the tiny index term survives only where d == 0
                dg_b, io_b = bass.broadcast_tensor_aps(
                    c3(dg), io[:].rearrange("x (a p) -> x a p", a=1)
                )
                gpsimd.tensor_tensor(
                    out=c3(keygs[i]), in0=io_b, in1=dg_b, op=Alu.add,
                )
                gpsimd.drain().then_inc(gkey, 1)
                gpsimd.engine_nop().then_inc(grel, 1)

    nc.compile()
    return nc


def _host_inputs(feats, trans):
    """Build per-core input dicts. feats [T,K] f32, trans [K,K] f32."""
    feats = np.ascontiguousarray(feats, dtype=np.float32)
    trans = np.ascontiguousarray(trans, dtype=np.float32)

    # exact fp32 prefix (reference arithmetic) for warm starts + drift rate
    v = np.full(K, NEG_INF, dtype=np.float32)
    v[START] = np.float32(0.0)
    vs = np.empty((P0 + 1, K), np.float32)
    vs[0] = v
    for t in range(P0):
        v = (v[None, :] + trans).max(axis=1) + feats[t]
        vs[t + 1] = v
    lam = float(vs[P0].max()) / P0

    ws = np.zeros((C, K), np.float32)
    for c in range(C):
        t0c = L * c - W
        if t0c < 0:
            ws[c] = 0.0
        elif t0c <= P0:
            ws[c] = vs[t0c]
        else:
            ws[c] = np.float32(lam * t0c)

    F = np.concatenate([np.zeros((W, K), np.float32), feats], axis=0)
    # lane c: warmup rows F[L*c : L*c+W], record rows F[L*c+W : L*c+W+L]
    sl = np.lib.stride_tricks.sliding_window_view(F, (W + L, K), axis=(0, 1))
    lanes = sl[np.arange(C) * L, 0]            # [C, W+L, K]
    featw_all = lanes[:, :W].reshape(C, W * K)
    featr_all = lanes[:, W:].reshape(C, L * K)

    init_v = np.full(K, NEG_INF, np.float32)
    init_v[START] = np.float32(0.0)

    transrep = np.broadcast_to(trans.reshape(1, K * K), (LPC, K * K))
    # packed index row: value at col p is p * EPS (broadcast over n on device)
    import ml_dtypes
    iotapm = np.broadcast_to(
        (_EPS * np.arange(K, dtype=np.float32)).astype(ml_dtypes.bfloat16).reshape(1, K),
        (LPC, K),
    )

    in_maps = []
    for k in range(NCORES):
        lanes_k = slice(k * LPC, (k + 1) * LPC)
        blend_k = np.full((LPC, K), np.float32(-65536.0))
        m1_k = np.zeros((LPC, 1), np.float32)
        if k == 0:
            blend_k[0] = init_v
            m1_k[0] = np.float32(-131072.0)
        in_maps.append({
            "warm": np.ascontiguousarray(ws[lanes_k]),
            "featw": np.ascontiguousarray(featw_all[lanes_k]),
            "featr": np.ascontiguousarray(featr_all[lanes_k]),
            "transrep": np.ascontiguousarray(transrep),
            "iotapm": np.ascontiguousarray(iotapm),
            "blend": blend_k,
            "m1": m1_k,
        })
    return in_maps


def _postprocess(feats, trans, bp_f32, vend_last):
    """bp_f32 [T, K] stored as index * EPS; vend_last [K] final lane state."""
    bp = np.rint(bp_f32.astype(np.float64) * float(2.0 ** 120)).astype(np.int32)
    terminal = vend_last + trans[STOP]
    best_last = int(np.argmax(terminal))

    # vectorized backtrack: compose within 16-step chunks, then chain chunks
    bpc = bp.reshape(C, L, K)
    comp = np.broadcast_to(np.arange(K, dtype=np.int32), (C, K)).copy()
    lane_i = np.arange(C)
    for s in range(L - 1, -1, -1):
        comp = bpc[lane_i[:, None], s, comp]   # comp[c, e] = state before step s
    # entry[c] = state at time 16c - 1 given exit state at time 16c+15
    exits = np.empty(C, np.int32)
    e = best_last
    for c in range(C - 1, -1, -1):
        exits[c] = e
        e = comp[c, e]
    path = np.empty((C, L), np.int32)
    cur = exits
    for s in range(L - 1, -1, -1):
        path[:, s] = cur
        cur = bpc[lane_i, s, cur]
    path = path.reshape(T)

    # reproduce the reference's fp32 left-fold score bitwise along the path
    prev = np.concatenate([[START], path[:-1]])
    tstep = trans[path, prev]                  # f32 [T]
    fstep = feats[np.arange(T), path]          # f32 [T]
    acc = np.float32(0.0)
    f32 = np.float32
    for a, b in zip(tstep.tolist(), fstep.tolist()):
        acc = f32(f32(acc + f32(a)) + f32(b))
    score = f32(acc + trans[STOP, path[-1]])
    return score, path


_last_results = None  # BassKernelResults of the most recent run (for profiling)


def kernel(feats, transitions):
    global _last_results
    feats = np.ascontiguousarray(feats, dtype=np.float32)
    trans = np.ascontiguousarray(transitions, dtype=np.float32)

    from concourse.bass_utils import run_bass_kernel_spmd

    nc = _build_nc()
    in_maps = _host_inputs(feats, trans)
    res = run_bass_kernel_spmd(nc, in_maps, list(range(NCORES)))
    _last_results = res

    bp_f32 = np.empty((T, K), np.float32)
    for k in range(NCORES):
        bp_k = np.asarray(res.results[k]["bp"]).reshape(LPC, L, K)
        t0k = 16 * (k * LPC)
        bp_f32[t0k:t0k + LPC * L] = bp_k.reshape(LPC * L, K)
    vend_last = np.asarray(res.results[NCORES - 1]["vend"])[LPC - 1]

    score, path = _postprocess(feats, trans, bp_f32, vend_last)
    return score, path.astype(np.int32)


# revision 34
# speedup vs baseline: 1.5309x; 1.0023x over previous
"""Viterbi (CRF decode) kernel for Trainium2, 8 NeuronCores.

Problem: single sequence T=16384, K=35 tags. reference computes a forward
Viterbi pass (max-plus recurrence over time, sequential), backpointers, then
backtracks; returns (path_score, best_path[T]).

Strategy: the time recurrence is chunked into 1024 lanes of 16 steps
(128 lanes per core x 8 cores, lane = one 16-step record window). Each lane
first runs a 32-step warmup from a magnitude-matched warm-start vector so
that, by tropical (max-plus) coalescence, its state equals the true forward
state up to a uniform additive constant that is a multiple of the local
float32 ulp grid -- which makes every subsequent fp32 add/max/argmax
bitwise-identical to the reference's (shift by a grid multiple commutes with
rounding within a binade). The record phase then emits exact backpointers
for its 16 time steps. Host side: a short exact prefix (512 steps) supplies
true warm starts for early lanes plus a drift estimate for the magnitude
guesses, and afterwards backtracks the backpointers and reproduces the
reference's fp32-accumulated path score bitwise by folding along the path.
"""

import numpy as np

K = 35
START = 33
STOP = 34
T = 16384
NEG_INF = np.float32(-10000.0)

W = 24          # warmup steps per lane
L = 16          # recorded steps per lane
P0 = 512        # host-exact prefix length
NCORES = 8
LPC = 128       # lanes per core
C = T // L      # 1024 lanes total

_BIG = float(2.0 ** 30)
_EPS = np.float32(2.0 ** -120)  # index packing scale: key = d + p*EPS


def _build_nc():
    import concourse.bass as bass
    import concourse.bacc as bacc
    import concourse.mybir as mybir

    f32 = mybir.dt.float32
    bf16 = mybir.dt.bfloat16
    Alu = mybir.AluOpType
    X = mybir.AxisListType.X

    nc = bacc.Bacc(detect_race_conditions=False)

    warm = nc.declare_dram_parameter("warm", [LPC, K], f32, isOutput=False)
    featw = nc.declare_dram_parameter("featw", [LPC, W * K], f32, isOutput=False)
    featr = nc.declare_dram_parameter("featr", [LPC, L * K], f32, isOutput=False)
    transrep = nc.declare_dram_parameter("transrep", [LPC, K * K], f32, isOutput=False)
    iotapm = nc.declare_dram_parameter("iotapm", [LPC, K], bf16, isOutput=False)
    blend = nc.declare_dram_parameter("blend", [LPC, K], f32, isOutput=False)
    m1 = nc.declare_dram_parameter("m1", [LPC, 1], f32, isOutput=False)
    bp_out = nc.declare_dram_parameter("bp", [LPC, L * K], f32, isOutput=True)
    vend_out = nc.declare_dram_parameter("vend", [LPC, K], f32, isOutput=True)

    XV = 10  # bp columns n < XV on Vector; n >= XV on GpSimd

    from contextlib import ExitStack

    with ExitStack() as ctx:
        tr = ctx.enter_context(nc.sbuf_tensor([LPC, K * K], f32))
        io = ctx.enter_context(nc.sbuf_tensor([LPC, K], bf16))
        fw = ctx.enter_context(nc.sbuf_tensor([LPC, W * K], f32))
        fr = ctx.enter_context(nc.sbuf_tensor([LPC, L * K], f32))
        bl = ctx.enter_context(nc.sbuf_tensor([LPC, K], f32))
        m1t = ctx.enter_context(nc.sbuf_tensor([LPC, 1], f32))
        v = ctx.enter_context(nc.sbuf_tensor([LPC, K], f32))
        newv_a = ctx.enter_context(nc.sbuf_tensor([LPC, K], f32))
        newv_b = ctx.enter_context(nc.sbuf_tensor([LPC, K], f32))
        cand_a = ctx.enter_context(nc.sbuf_tensor([LPC, K * K], f32))
        cand_b = ctx.enter_context(nc.sbuf_tensor([LPC, K * K], f32))
        dg = ctx.enter_context(nc.sbuf_tensor([LPC, K * K], bf16))
        keyg_a = ctx.enter_context(nc.sbuf_tensor([LPC, K * K], bf16))
        keyg_b = ctx.enter_context(nc.sbuf_tensor([LPC, K * K], bf16))
        bpt = ctx.enter_context(nc.sbuf_tensor([LPC, L * K], f32))
        dma_sem = ctx.enter_context(nc.semaphore())
        v_sem = ctx.enter_context(nc.semaphore())
        vsem = ctx.enter_context(nc.semaphore())   # vector -> gpsimd: step ready
        grel = ctx.enter_context(nc.semaphore())   # gpsimd -> vector: cand/newv consumed
        gkey = ctx.enter_context(nc.semaphore())   # gpsimd -> vector: key ready
        block = ctx.enter_context(nc.Block())
        cands = [cand_a, cand_b]
        newvs = [newv_a, newv_b]
        keygs = [keyg_a, keyg_b]

        def c3(t, lo=0, hi=K):
            return t[:, lo * K:hi * K].rearrange("x (n p) -> x n p", n=hi - lo)

        @block.sync
        def _(sync):
            for dst, src in ((v, warm), (fw, featw)):
                sync.dma_start(out=dst[:], in_=src[:]).then_inc(dma_sem, 16)
            sync.wait_ge(v_sem, 1)
            sync.dma_start(out=bp_out[:], in_=bpt[:]).then_inc(dma_sem, 16)
            sync.dma_start(out=vend_out[:], in_=v[:]).then_inc(dma_sem, 16)

        @block.scalar
        def _(scalar):
            scalar.dma_start(out=tr[:], in_=transrep[:]).then_inc(dma_sem, 16)
            for dst, src in ((fr, featr), (bl, blend), (m1t, m1), (io, iotapm)):
                scalar.dma_start(out=dst[:], in_=src[:]).then_inc(dma_sem, 16)

        @block.vector
        def _(vector):
            vector.wait_ge(dma_sem, 3 * 16)

            def rmin(rec_s):
                # stored bp[rec_s] = min over p of (d + p*EPS) = first_argmax * EPS
                vector.wait_ge(gkey, rec_s + 1)
                vector.tensor_reduce(
                    out=bpt[:, rec_s * K:(rec_s + 1) * K],
                    in_=c3(keygs[rec_s % 2]),
                    axis=X, op=Alu.min,
                )

            def step(feat_slice, rec_s):
                i = (rec_s or 0) % 2
                cand, newv = (cands[i], newvs[i]) if rec_s is not None else (cands[0], newvs[0])
                if rec_s is not None and rec_s >= 2:
                    # gpsimd must be done reading cand/newv of step rec_s-2
                    vector.wait_ge(grel, rec_s - 1)
                cand3 = c3(cand)
                v_b, tr_b = bass.broadcast_tensor_aps(
                    v[:].rearrange("x (a m) -> x a m", a=1),
                    tr[:].rearrange("x (n p) -> x n p", n=K),
                )
                vector.tensor_tensor(out=cand3, in0=v_b, in1=tr_b, op=Alu.add)
                r = vector.tensor_reduce(out=newv[:], in_=cand3, axis=X, op=Alu.max)
                if rec_s is not None:
                    vector.drain().then_inc(vsem, 1)
                vector.tensor_tensor(out=v[:], in0=newv[:], in1=feat_slice, op=Alu.add)
                if rec_s is not None and rec_s >= 1:
                    rmin(rec_s - 1)   # previous step's argmin
                vector.drain()

            for s in range(W):
                step(fw[:, s * K:(s + 1) * K], None)

            # blend: record start = max(v + m1, blend_tile); on core 0 lane 0
            # m1 = -2^17 and blend = init_v, forcing the true initial vector.
            vector.wait_ge(dma_sem, 7 * 16)
            vector.tensor_scalar_add(out=newv_a[:], in0=v[:], scalar1=m1t[:, 0:1])
            vector.drain()
            vector.tensor_tensor(out=v[:], in0=newv_a[:], in1=bl[:], op=Alu.max)
            vector.drain()

            for s in range(L):
                step(fr[:, s * K:(s + 1) * K], s)

            rmin(L - 1)
            vector.drain().then_inc(v_sem, 1)

        @block.gpsimd
        def _(gpsimd):
            from concourse import library_config
            gpsimd.load_library(library_config.standard)
            for s in range(L):
                i = s % 2
                cand, newv = cands[i], newvs[i]
                gpsimd.wait_ge(vsem, s + 1)
                # d[n, p] = newv[n] - cand[n, p]  (exactly 0.0 at the argmax set)
                nv_b, cand_b_ = bass.broadcast_tensor_aps(
                    newv[:].rearrange("x (m a) -> x m a", a=1), c3(cand)
                )
                gpsimd.tensor_tensor(
                    out=c3(dg), in0=nv_b, in1=cand_b_, op=Alu.subtract,
                )
                # key = d + p*EPS: the tiny index term survives only where d == 0
                dg_b, io_b = bass.broadcast_tensor_aps(
                    c3(dg), io[:].rearrange("x (a p) -> x a p", a=1)
                )
                gpsimd.tensor_tensor(
                    out=c3(keygs[i]), in0=io_b, in1=dg_b, op=Alu.add,
                )
                gpsimd.drain().then_inc(gkey, 1)
                gpsimd.engine_nop().then_inc(grel, 1)

    nc.compile()
    return nc


def _host_inputs(feats, trans):
    """Build per-core input dicts. feats [T,K] f32, trans [K,K] f32."""
    feats = np.ascontiguousarray(feats, dtype=np.float32)
    trans = np.ascontiguousarray(trans, dtype=np.float32)

    # exact fp32 prefix (reference arithmetic) for warm starts + drift rate
    v = np.full(K, NEG_INF, dtype=np.float32)
    v[START] = np.float32(0.0)
    vs = np.empty((P0 + 1, K), np.float32)
    vs[0] = v
    for t in range(P0):
        v = (v[None, :] + trans).max(axis=1) + feats[t]
        vs[t + 1] = v
    lam = float(vs[P0].max()) / P0

    ws = np.zeros((C, K), np.float32)
    for c in range(C):
        t0c = L * c - W
        if t0c < 0:
            ws[c] = 0.0
        elif t0c <= P0:
            ws[c] = vs[t0c]
        else:
            ws[c] = np.float32(lam * t0c)

    F = np.concatenate([np.zeros((W, K), np.float32), feats], axis=0)
    # lane c: warmup rows F[L*c : L*c+W], record rows F[L*c+W : L*c+W+L]
    sl = np.lib.stride_tricks.sliding_window_view(F, (W + L, K), axis=(0, 1))
    lanes = sl[np.arange(C) * L, 0]            # [C, W+L, K]
    featw_all = lanes[:, :W].reshape(C, W * K)
    featr_all = lanes[:, W:].reshape(C, L * K)

    init_v = np.full(K, NEG_INF, np.float32)
    init_v[START] = np.float32(0.0)

    transrep = np.broadcast_to(trans.reshape(1, K * K), (LPC, K * K))
    # packed index row: value at col p is p * EPS (broadcast over n on device)
    import ml_dtypes
    iotapm = np.broadcast_to(
        (_EPS * np.arange(K, dtype=np.float32)).astype(ml_dtypes.bfloat16).reshape(1, K),
        (LPC, K),
    )

    in_maps = []
    for k in range(NCORES):
        lanes_k = slice(k * LPC, (k + 1) * LPC)
        blend_k = np.full((LPC, K), np.float32(-65536.0))
        m1_k = np.zeros((LPC, 1), np.float32)
        if k == 0:
            blend_k[0] = init_v
            m1_k[0] = np.float32(-131072.0)
        in_maps.append({
            "warm": np.ascontiguousarray(ws[lanes_k]),
            "featw": np.ascontiguousarray(featw_all[lanes_k]),
            "featr": np.ascontiguousarray(featr_all[lanes_k]),
            "transrep": np.ascontiguousarray(transrep),
            "iotapm": np.ascontiguousarray(iotapm),
            "blend": blend_k,
            "m1": m1_k,
        })
    return in_maps


def _postprocess(feats, trans, bp_f32, vend_last):
    """bp_f32 [T, K] stored as index * EPS; vend_last [K] final lane state."""
    bp = np.rint(bp_f32.astype(np.float64) * float(2.0 ** 120)).astype(np.int32)
    terminal = vend_last + trans[STOP]
    best_last = int(np.argmax(terminal))

    # vectorized backtrack: compose within 16-step chunks, then chain chunks
    bpc = bp.reshape(C, L, K)
    comp = np.broadcast_to(np.arange(K, dtype=np.int32), (C, K)).copy()
    lane_i = np.arange(C)
    for s in range(L - 1, -1, -1):
        comp = bpc[lane_i[:, None], s, comp]   # comp[c, e] = state before step s
    # entry[c] = state at time 16c - 1 given exit state at time 16c+15
    exits = np.empty(C, np.int32)
    e = best_last
    for c in range(C - 1, -1, -1):
        exits[c] = e
        e = comp[c, e]
    path = np.empty((C, L), np.int32)
    cur = exits
    for s in range(L - 1, -1, -1):
        path[:, s] = cur
        cur = bpc[lane_i, s, cur]
    path = path.reshape(T)

    # reproduce the reference's fp32 left-fold score bitwise along the path
    prev = np.concatenate([[START], path[:-1]])
    tstep = trans[path, prev]                  # f32 [T]
    fstep = feats[np.arange(T), path]          # f32 [T]
    acc = np.float32(0.0)
    f32 = np.float32
    for a, b in zip(tstep.tolist(), fstep.tolist()):
        acc = f32(f32(acc + f32(a)) + f32(b))
    score = f32(acc + trans[STOP, path[-1]])
    return score, path


_last_results = None  # BassKernelResults of the most recent run (for profiling)


def kernel(feats, transitions):
    global _last_results
    feats = np.ascontiguousarray(feats, dtype=np.float32)
    trans = np.ascontiguousarray(transitions, dtype=np.float32)

    from concourse.bass_utils import run_bass_kernel_spmd

    nc = _build_nc()
    in_maps = _host_inputs(feats, trans)
    res = run_bass_kernel_spmd(nc, in_maps, list(range(NCORES)))
    _last_results = res

    bp_f32 = np.empty((T, K), np.float32)
    for k in range(NCORES):
        bp_k = np.asarray(res.results[k]["bp"]).reshape(LPC, L, K)
        t0k = 16 * (k * LPC)
        bp_f32[t0k:t0k + LPC * L] = bp_k.reshape(LPC * L, K)
    vend_last = np.asarray(res.results[NCORES - 1]["vend"])[LPC - 1]

    score, path = _postprocess(feats, trans, bp_f32, vend_last)
    return score, path.astype(np.int32)
